# revision 44
# baseline (speedup 1.0000x reference)
"""Trainium2 Bass kernel for nn_LinkPredictor (2-layer GCN + edge-dot decode).

Strategy (8 NeuronCores, SPMD), v4:
  - Nodes sharded: core c owns rows [c*12544, (c+1)*12544) of the padded
    node table (N=100000 padded to 100352 = 8*98*128).
  - dinv folded into node features: table rows hold hs = dinv[n] * (prev @ W);
    output z = relu(dinv[v]*(agg + hs[v]) + b).
  - Node tables in DRAM are band-major contiguous: collective chunk K holds
    rows c*CH+j of each core's shard; 4 pipelined AllGathers per layer write
    slices of one tensor. Chunk sizes [14,32,32,20] windows: small first
    chunk so layer-1 gathers start early, small last chunk for a short
    decode tail.
  - Gathers use int16 indices relative to per-band bases; band == chunk.
    One dma_gather per (window-batch, gather-band), WB=4 windows per batch.
  - Aggregation: one-hot S built in WIDE batched DVE ops (one tensor_tensor
    is_equal per (batch, band) span using stride-0 broadcast APs) feeding
    PE matmul accumulation into PSUM. Self-loop = identity matmul; relu+dinv
    scale on ScalarE.
  - hs tiles resident in SBUF as wide [128, 12544] tiles; layer-2 output z
    aliases the layer-1 hs tile.
  - Next layer's first batches are prefetched during the current layer's
    tail through a shared M pool.
  - Decode: gathers z[s], z[d] by gather-band pair, one DVE
    tensor_tensor_reduce (mult+add) per tile.
"""
import contextlib
import math
import numpy as np
import ml_dtypes

import concourse.bass as bass
import concourse.tile as tile
from concourse import bacc, mybir
from concourse.bass_utils import run_bass_kernel_spmd
from concourse.tile_rust import add_dep_helper

F32 = mybir.dt.float32
BF16 = mybir.dt.bfloat16
I16 = mybir.dt.int16
BF = ml_dtypes.bfloat16
ACTF = mybir.ActivationFunctionType


class Cfg:
    def __init__(self, N=100000, E=1600000, EL=100000, D=128, ncores=8,
                 nw=98, wb=3, prefetch=3, ch_win=(16, 16, 14, 20, 18, 14),
                 band_split=3):
        self.N, self.E, self.EL, self.D, self.NC = N, E, EL, D, ncores
        self.NW = nw                      # windows (128 nodes each) per core
        self.SHARD = nw * 128             # nodes per core (padded)
        self.NP = self.SHARD * ncores     # padded node count
        assert self.NP >= N
        # collective chunks (rows per core), window-aligned; first chunk
        # small so the first AllGather (and layer-1 gathers) start early,
        # last chunk small so the decode tail is short.
        self.CH_WIN = list(ch_win)        # windows per chunk
        assert sum(self.CH_WIN) == nw
        self.NB = len(self.CH_WIN)
        self.CH_SIZES = [wn * 128 for wn in self.CH_WIN]
        self.CH_STARTS = np.cumsum([0] + self.CH_SIZES).tolist()
        self.TB_SIZES = [s * ncores for s in self.CH_SIZES]
        self.TB_STARTS = np.cumsum([0] + self.TB_SIZES).tolist()
        # 2 gather bands of <=65536 table rows each (signed int16 indices
        # relative to a mid-band base; hardware sign-extends). Band g covers
        # collective chunks GB_CHUNKS[g]; many small chunks keep each
        # AllGather short so layer boundaries don't stall on a big one.
        self.NGB = 2
        bs = self.BAND_SPLIT = band_split
        self.GB_CHUNKS = [list(range(bs)), list(range(bs, self.NB))]
        self.BAND_LO = [self.TB_STARTS[0], self.TB_STARTS[bs]]
        self.BAND_HI = [self.TB_STARTS[bs], self.TB_STARTS[self.NB]]
        for g in range(self.NGB):
            assert self.BAND_HI[g] - self.BAND_LO[g] <= 65536
        self.GBASE = [self.BAND_LO[g] + 32768 for g in range(self.NGB)]
        self.WB = wb                      # windows per gather/aggregate batch
        self.NBATCH = math.ceil(nw / wb)
        self.PREFETCH = prefetch          # next-layer batches gathered early


DEFAULT = Cfg()


def _wrap_idxs(idx):
    """[n] ints -> [128, n//16] int16 wrapped in 16 partitions, replicated 8x."""
    n = len(idx)
    assert n % 16 == 0
    w = np.asarray(idx, dtype=np.int16).reshape(n // 16, 16).T
    return np.ascontiguousarray(np.tile(w, (8, 1)))


def host_prep(cfg, x, edge_index, edge_label_index, W1, b1, W2, b2):
    """All host-side sharding/layout. Returns (per-core input maps, meta)."""
    c = cfg
    src = np.asarray(edge_index[0], dtype=np.int64)
    dst = np.asarray(edge_index[1], dtype=np.int64)
    deg = np.bincount(dst, minlength=c.N).astype(np.float64) + 1.0
    dinv = 1.0 / np.sqrt(deg)                      # [N]
    dinv_p = np.ones(c.NP, dtype=np.float64)
    dinv_p[:c.N] = dinv
    dinv_f = dinv_p.astype(np.float32)

    ch_starts = np.asarray(c.CH_STARTS[:-1])
    def chunk_of(r):
        return np.searchsorted(ch_starts, r, side="right") - 1

    def bmaj_of(n):
        """band-major table row of node id n."""
        cc, r = n // c.SHARD, n % c.SHARD
        K = chunk_of(r)
        return (np.asarray(c.TB_STARTS)[K] + cc * np.asarray(c.CH_SIZES)[K]
                + (r - ch_starts[K]))

    bmaj_src = bmaj_of(src)
    gband_src = (chunk_of(src % c.SHARD) >= c.BAND_SPLIT).astype(np.int64)
    core_of = dst // c.SHARD
    w_of = (dst % c.SHARD) // 128
    dloc = dst % 128

    key = (core_of * c.NW + w_of) * c.NGB + gband_src
    ngroups = c.NC * c.NW * c.NGB
    order = np.argsort(key, kind="stable")
    counts = np.bincount(key, minlength=ngroups).reshape(c.NC, c.NW, c.NGB)
    starts = np.zeros(ngroups + 1, dtype=np.int64)
    np.cumsum(np.bincount(key, minlength=ngroups), out=starts[1:])

    T = np.ceil(counts.max(axis=0) / 128).astype(np.int64)     # [NW, NGB]
    TOT_TILES = int(T.sum())
    span_tiles = np.zeros((c.NBATCH, c.NGB), dtype=np.int64)
    for b in range(c.NBATCH):
        wlo, whi = b * c.WB, min((b + 1) * c.WB, c.NW)
        for g in range(c.NGB):
            span_tiles[b, g] = T[wlo:whi, g].sum()
    TOT = TOT_TILES * 128

    idx_arr = np.zeros((c.NC, TOT), dtype=np.int64)
    dloc_arr = np.full((c.NC, TOT), -1.0, dtype=np.float32)
    for core in range(c.NC):
        pos = 0
        for b in range(c.NBATCH):
            wlo, whi = b * c.WB, min((b + 1) * c.WB, c.NW)
            for g in range(c.NGB):
                grp_start = pos
                for w in range(wlo, whi):
                    gk = (core * c.NW + w) * c.NGB + g
                    eids = order[starts[gk]:starts[gk + 1]]
                    n = len(eids)
                    idx_arr[core, pos:pos + n] = bmaj_src[eids] - c.GBASE[g]
                    dloc_arr[core, pos:pos + n] = dloc[eids]
                    grp_start = pos
                    pos += int(T[w, g]) * 128
                # the ucode strips TRAILING negative indices from a gather:
                # the span's final slot must be >= 0. Swap within the last
                # window group (slots there share the same dst window).
                if pos > 0 and idx_arr[core, pos - 1] < 0:
                    cand = np.nonzero(idx_arr[core, grp_start:pos] >= 0)[0]
                    assert len(cand) > 0, "all-negative final group"
                    j = grp_start + cand[0]
                    for arr in (idx_arr, dloc_arr):
                        arr[core, j], arr[core, pos - 1] = \
                            arr[core, pos - 1], arr[core, j]
        assert pos == TOT
    assert idx_arr.min() >= -32768 and idx_arr.max() < 32768

    # decode: label edge j -> core j // ELC; groups by (gband(s), gband(d))
    assert c.EL % c.NC == 0
    ELC = c.EL // c.NC
    ls = np.asarray(edge_label_index[0], dtype=np.int64)
    ld = np.asarray(edge_label_index[1], dtype=np.int64)
    bs, bd = bmaj_of(ls), bmaj_of(ld)
    gs = (chunk_of(ls % c.SHARD) >= c.BAND_SPLIT).astype(np.int64)
    gd = (chunk_of(ld % c.SHARD) >= c.BAND_SPLIT).astype(np.int64)
    gdec = gs * c.NGB + gd
    NG_DEC = c.NGB * c.NGB
    cnt_dec = np.zeros((c.NC, NG_DEC), dtype=np.int64)
    for core in range(c.NC):
        cnt_dec[core] = np.bincount(gdec[core * ELC:(core + 1) * ELC],
                                    minlength=NG_DEC)
    Tdec = np.ceil(cnt_dec.max(axis=0) / 128).astype(np.int64)   # [NG_DEC]
    gorder = sorted(range(NG_DEC), key=lambda g: (max(g // c.NGB, g % c.NGB), g))
    TOT_DEC = int(Tdec.sum()) * 128
    idx_s = np.zeros((c.NC, TOT_DEC), dtype=np.int64)
    idx_d = np.zeros((c.NC, TOT_DEC), dtype=np.int64)
    slot2j = np.full((c.NC, TOT_DEC), -1, dtype=np.int64)
    for core in range(c.NC):
        jlo = core * ELC
        kk = gdec[jlo:jlo + ELC]
        o = np.argsort(kk, kind="stable")
        st = np.zeros(NG_DEC + 1, dtype=np.int64)
        np.cumsum(np.bincount(kk, minlength=NG_DEC), out=st[1:])
        pos = 0
        for g in gorder:
            js = o[st[g]:st[g + 1]] + jlo
            n = len(js)
            idx_s[core, pos:pos + n] = bs[js] - c.GBASE[g // c.NGB]
            idx_d[core, pos:pos + n] = bd[js] - c.GBASE[g % c.NGB]
            slot2j[core, pos:pos + n] = js
            p1 = pos + int(Tdec[g]) * 128
            # keep the final slot of each decode gather span non-negative
            # (the ucode strips trailing negative indices)
            if p1 > pos and (idx_s[core, p1 - 1] < 0 or
                             idx_d[core, p1 - 1] < 0):
                ok = np.nonzero((idx_s[core, pos:p1] >= 0) &
                                (idx_d[core, pos:p1] >= 0))[0]
                assert len(ok) > 0
                j = pos + ok[0]
                for a2 in (idx_s, idx_d, slot2j):
                    a2[core, j], a2[core, p1 - 1] = \
                        a2[core, p1 - 1], a2[core, j]
            pos = p1
        assert pos == TOT_DEC

    xp = np.zeros((c.NP, c.D), dtype=np.float32)
    xp[:c.N] = np.asarray(x, dtype=np.float32)
    use_b1 = bool(np.any(np.asarray(b1)))
    use_b2 = bool(np.any(np.asarray(b2)))
    assert not (use_b1 or use_b2), "bias path not wired in v10"

    # xs = dinv * x, in band-major table order: layer-1 gathers read this
    # host-provided table directly -> no P0 / no table-1 collectives.
    xs = xp * dinv_f[:, None]
    perm = bmaj_of(np.arange(c.NP))        # node id -> table row
    xs_tab = np.zeros((c.NP, c.D), dtype=np.float32)
    xs_tab[perm] = xs
    xs_tab = np.ascontiguousarray(xs_tab).astype(BF)

    in_maps = []
    for core in range(c.NC):
        sl = slice(core * c.SHARD, (core + 1) * c.SHARD)
        dsh = dinv_f[sl]
        m = {
            "xs_tab": xs_tab,
            "xs_own": np.ascontiguousarray(
                xs[sl].reshape(c.NW, 128, c.D).transpose(1, 0, 2)
                .reshape(128, c.NW * c.D)).astype(BF),
            "W1": np.asarray(W1, dtype=np.float32).astype(BF),
            "W2": np.asarray(W2, dtype=np.float32).astype(BF),
            "dinv": np.ascontiguousarray(dsh.reshape(c.NW, 128).T),
            "gidx": _wrap_idxs(idx_arr[core]),
            "dstloc": np.ascontiguousarray(
                dloc_arr[core].reshape(TOT_TILES, 128).T).astype(BF),
            "didx_s": _wrap_idxs(idx_s[core]),
            "didx_d": _wrap_idxs(idx_d[core]),
        }
        in_maps.append(m)
    meta = dict(T=T, span_tiles=span_tiles, TOT=TOT, TOT_TILES=TOT_TILES,
                Tdec=Tdec, gorder=gorder, TOT_DEC=TOT_DEC, slot2j=slot2j,
                use_b1=use_b1, use_b2=use_b2)
    return in_maps, meta


def build_program(cfg, meta, num_cores=None):
    c = cfg
    NCores = num_cores or c.NC
    T, span_tiles = meta["T"], meta["span_tiles"]
    TOT, TOT_TILES = meta["TOT"], meta["TOT_TILES"]
    Tdec, gorder, TOT_DEC = meta["Tdec"], meta["gorder"], meta["TOT_DEC"]
    use_b = {1: meta["use_b1"], 2: meta["use_b2"]}
    D = c.D
    TBMAX = int(span_tiles.sum(axis=1).max())
    SPANMAX = int(span_tiles.max())
    GB_CHUNKS = c.GB_CHUNKS

    nc = bacc.Bacc("TRN2", target_bir_lowering=False, debug=False,
                   num_devices=NCores, num_swdge_queues=4)
    NQ = 4

    assert not (use_b[1] or use_b[2])
    xs_tab_in = nc.dram_tensor("xs_tab", [c.TB_STARTS[-1], D], BF16,
                               kind="ExternalInput")
    xs_own_in = nc.dram_tensor("xs_own", [128, c.SHARD], BF16,
                               kind="ExternalInput")
    W1_in = nc.dram_tensor("W1", [D, D], BF16, kind="ExternalInput")
    W2_in = nc.dram_tensor("W2", [D, D], BF16, kind="ExternalInput")
    dinv_in = nc.dram_tensor("dinv", [128, c.NW], F32, kind="ExternalInput")
    gidx_in = nc.dram_tensor("gidx", [128, TOT // 16], I16, kind="ExternalInput")
    dstloc_in = nc.dram_tensor("dstloc", [128, TOT_TILES], BF16, kind="ExternalInput")
    didx_s_in = nc.dram_tensor("didx_s", [128, TOT_DEC // 16], I16, kind="ExternalInput")
    didx_d_in = nc.dram_tensor("didx_d", [128, TOT_DEC // 16], I16, kind="ExternalInput")
    dots_out = nc.dram_tensor("dots", [128, TOT_DEC // 128], F32, kind="ExternalOutput")

    shard_b = {l: [nc.dram_tensor(f"shard{l}_{k}", [c.CH_SIZES[k], D], BF16)
                   for k in range(c.NB)] for l in (2, 3)}
    table = {l: nc.dram_tensor(f"table{l}", [c.TB_STARTS[-1], D], BF16,
                               addr_space="Shared") for l in (2, 3)}

    def tslice(l, g):
        """Gather source AP for band g: base at GBASE[g] (mid-band); signed
        int16 indices reach the whole band [BAND_LO, BAND_HI). Layer 1
        gathers the host-provided xs table (no collective needed)."""
        src = xs_tab_in if l == 1 else table[l]
        return src[c.GBASE[g]:c.BAND_HI[g], :]

    iota_dram = nc.inline_tensor(
        np.tile(np.arange(128, dtype=np.float32), (128, 1)).astype(BF), "iota_c")
    ident_dram = nc.inline_tensor(np.eye(128, dtype=np.float32).astype(BF), "ident_c")

    core_ids = list(range(NCores))
    gst = {"count": 0, "prev": None}
    ccst = {}                            # (l, K) -> collective instruction

    def emit_gather(out_ap, in_ap, idx_ap, n_idx, deps=()):
        q = gst["count"] % NQ
        inst = nc.gpsimd.dma_gather(out_ap, in_ap, idx_ap, n_idx, n_idx, D,
                                    queue_num=q, single_packet=False)
        for dcc in deps:
            add_dep_helper(inst.ins, dcc.ins, sync=True,
                           reason="gather after collective")
        gst["count"] += 1
        return inst

    def emit_collective(l, K):
        cc = nc.gpsimd.collective_compute(
            "AllGather", mybir.AluOpType.bypass,
            replica_groups=[core_ids],
            ins=[shard_b[l][K][:]],
            outs=[table[l][c.TB_STARTS[K]:c.TB_STARTS[K + 1], :]],
        )
        ccst[(l, K)] = cc
        return cc

    def write_window(l, w, src_ap, done_k):
        """DMA window w rows into its chunk shard; fire collectives when a
        chunk completes (chunks are window-aligned)."""
        lo = w * 128
        K = 0
        while c.CH_STARTS[K + 1] <= lo:
            K += 1
        off = lo - c.CH_STARTS[K]
        nc.sync.dma_start(shard_b[l][K][off:off + 128, :], src_ap)
        while len(done_k) < c.NB and \
                (w + 1) * 128 >= c.CH_STARTS[len(done_k) + 1]:
            emit_collective(l, len(done_k))
            done_k.append(len(done_k))

    with tile.TileContext(nc) as tc:
        with contextlib.ExitStack() as es:
            const = es.enter_context(tc.tile_pool(name="const", bufs=1))
            meta_p = es.enter_context(tc.tile_pool(name="meta", bufs=1))

            w1_sb = const.tile([D, D], BF16); nc.sync.dma_start(w1_sb[:], W1_in[:])
            w2_sb = const.tile([D, D], BF16); nc.sync.dma_start(w2_sb[:], W2_in[:])
            dinv_sb = const.tile([128, c.NW], F32)
            nc.sync.dma_start(dinv_sb[:], dinv_in[:])
            iota_sb = const.tile([128, 128], BF16)
            nc.sync.dma_start(iota_sb[:], iota_dram[:])
            ident_sb = const.tile([128, 128], BF16)
            nc.sync.dma_start(ident_sb[:], ident_dram[:])
            # gidx first: the first gathers need it; dstloc/xs_own follow on
            # other queues
            gidx_sb = meta_p.tile([128, TOT // 16], I16)
            nc.scalar.dma_start(gidx_sb[:], gidx_in[:])
            dstloc_sb = meta_p.tile([128, TOT_TILES], BF16)
            nc.scalar.dma_start(dstloc_sb[:], dstloc_in[:])
            xs_own_sb = meta_p.tile([128, c.SHARD], BF16)
            nc.sync.dma_start(xs_own_sb[:], xs_own_in[:])

            span_base = {}
            tcol0 = 0
            for b in range(c.NBATCH):
                m0 = 0
                for g in range(c.NGB):
                    span_base[(b, g)] = (tcol0, m0)
                    tcol0 += int(span_tiles[b, g])
                    m0 += int(span_tiles[b, g])

            pre = {}        # (lid, b) -> (dict g -> Mt, set of emitted gbands)
            cc_waited = {1: set(), 2: set(), 3: set()}
            SPANG = [int(span_tiles[:, g].max()) for g in range(c.NGB)]

            def batch_gathers(Mp, lid, b, only_avail=False):
                """Emit (remaining) gathers for batch b of layer lid. Per-band
                M pools: band A (early-available) pipelines deeply without
                waiting for band B's collectives."""
                Mts, done = pre.get((lid, b), ({}, set()))
                for g in range(c.NGB):
                    if g in done or int(span_tiles[b, g]) == 0:
                        continue
                    if lid > 1 and only_avail and any((lid, K) not in ccst
                                                      for K in GB_CHUNKS[g]):
                        continue
                    ntiles = int(span_tiles[b, g])
                    tb, _ = span_base[(b, g)]
                    deps = []
                    if lid > 1 and g not in cc_waited[lid]:
                        deps = [ccst[(lid, K)] for K in GB_CHUNKS[g]]
                        cc_waited[lid].add(g)
                    Mt = Mp.tile([128, SPANG[g], 128], BF16, tag=f"M{g}",
                                 bufs=(8 if g == 0 else 3))
                    emit_gather(
                        Mt[:, :ntiles, :], tslice(lid, g),
                        gidx_sb[:, tb * 8:(tb + ntiles) * 8],
                        ntiles * 128, deps=deps)
                    Mts[g] = Mt
                    done.add(g)
                pre[(lid, b)] = (Mts, done)
                return Mts

            def build_spans(Sp, b):
                """One wide DVE is_equal per (batch, band) span: all one-hot
                S tiles of the span in a single instruction."""
                Sw = {}
                for g in range(c.NGB):
                    K = int(span_tiles[b, g])
                    if K == 0:
                        continue
                    tb, mb = span_base[(b, g)]
                    St = Sp.tile([128, SPANG[g], 128], BF16, tag=f"S{g}",
                                 bufs=3)
                    nc.vector.tensor_tensor(
                        St[:, :K, :],
                        iota_sb[:].unsqueeze(1).broadcast_to([128, K, 128]),
                        dstloc_sb[:, tb:tb + K].unsqueeze(2)
                            .broadcast_to([128, K, 128]),
                        op=mybir.AluOpType.is_equal)
                    Sw[g] = (St, mb)
                return Sw

            def layer(Mp, lid, hall, hall_next, next_lid):
                """lid==1: aggregate raw xs, then apply W1 (transpose+matmul)
                and W2 per window to produce hs2. lid==2: aggregate hs2,
                relu+scale to produce z."""
                waited_done = []
                with tc.tile_pool(name=f"S{lid}", bufs=6) as Sp, \
                     tc.tile_pool(name=f"ag{lid}", bufs=4, space="PSUM") as agp, \
                     tc.tile_pool(name=f"tp{lid}", bufs=1, space="PSUM") as tpp, \
                     tc.tile_pool(name=f"ep{lid}", bufs=4) as epp:
                    for b in range(c.NBATCH):
                        wlo, whi = b * c.WB, min((b + 1) * c.WB, c.NW)
                        Mts = batch_gathers(Mp, lid, b)
                        Sw = build_spans(Sp, b)
                        for w in range(wlo, whi):
                            ps = agp.tile([128, D], F32, tag="agg")
                            nmm = int(T[w].sum())
                            hsl = hall[:, w * 128:w * 128 + D]
                            nc.tensor.matmul(ps[:], lhsT=ident_sb[:],
                                             rhs=hsl,
                                             start=True, stop=(nmm == 0))
                            mi = 0
                            for g in range(c.NGB):
                                if int(T[w, g]) == 0:
                                    continue
                                St, _ = Sw[g]
                                Mt = Mts[g]
                                off = int(T[wlo:w, g].sum())
                                for t in range(int(T[w, g])):
                                    k = off + t
                                    mi += 1
                                    nc.tensor.matmul(
                                        ps[:], lhsT=St[:, k, :],
                                        rhs=Mt[:, k, :],
                                        start=False,
                                        stop=(mi == nmm))
                            zo = None
                            if lid == 1:
                                # a1 = dinv*(agg+self)  [pre-W1 aggregate]
                                a1 = epp.tile([128, D], BF16, tag="a1")
                                nc.scalar.activation(
                                    a1[:], ps[:], ACTF.Copy,
                                    scale=dinv_sb[:, w:w + 1])
                                t1ps = tpp.tile([128, D], BF16, tag="t1")
                                nc.tensor.transpose(t1ps[:], a1[:], ident_sb[:])
                                a1T = epp.tile([128, D], BF16, tag="a1T")
                                nc.scalar.activation(a1T[:], t1ps[:], ACTF.Copy)
                                yps = tpp.tile([128, D], F32, tag="y")
                                nc.tensor.matmul(yps[:], lhsT=a1T[:],
                                                 rhs=w1_sb[:],
                                                 start=True, stop=True)
                                z = epp.tile([128, D], BF16, tag="z")
                                nc.scalar.activation(z[:], yps[:], ACTF.Relu)
                                t2ps = tpp.tile([128, D], BF16, tag="t2")
                                nc.tensor.transpose(t2ps[:], z[:], ident_sb[:])
                                zT = epp.tile([128, D], BF16, tag="zT")
                                nc.scalar.activation(zT[:], t2ps[:], ACTF.Copy)
                                h2ps = tpp.tile([128, D], F32, tag="h2")
                                nc.tensor.matmul(h2ps[:], lhsT=zT[:],
                                                 rhs=w2_sb[:],
                                                 start=True, stop=True)
                                nc.scalar.activation(
                                    hall_next[:, w * 128:w * 128 + D], h2ps[:],
                                    ACTF.Copy, scale=dinv_sb[:, w:w + 1])
                            else:
                                # layer-2 output only stages toward the shard
                                # DMA; use a small rolling buffer
                                zo = epp.tile([128, D], BF16, tag="zo")
                                nc.scalar.activation(
                                    zo[:], ps[:],
                                    ACTF.Relu, scale=dinv_sb[:, w:w + 1])
                            out_ap = (hall_next[:, w * 128:w * 128 + D]
                                      if lid == 1 else zo[:])
                            write_window(next_lid, w, out_ap, waited_done)
                        if b == c.NBATCH - 1 - c.PREFETCH and next_lid == 2:
                            for pb in range(min(c.PREFETCH + 1, c.NBATCH)):
                                batch_gathers(Mp, 2, pb, only_avail=True)

            with tc.tile_pool(name="hs", bufs=1) as hsp, \
                 tc.tile_pool(name="Mpool", bufs=1) as Mp:
                hs2_all = hsp.tile([128, c.SHARD], BF16, tag="hs2")
                layer(Mp, 1, xs_own_sb, hs2_all, 2)
                layer(Mp, 2, hs2_all, None, 3)

            # decode
            with tc.tile_pool(name="didx", bufs=1) as didxp, \
                 tc.tile_pool(name="dM", bufs=1) as dMp, \
                 tc.tile_pool(name="dw", bufs=6) as dwp, \
                 tc.tile_pool(name="dout", bufs=1) as doutp:
                ds_sb = didxp.tile([128, TOT_DEC // 16], I16)
                nc.scalar.dma_start(ds_sb[:], didx_s_in[:])
                dd_sb = didxp.tile([128, TOT_DEC // 16], I16)
                nc.scalar.dma_start(dd_sb[:], didx_d_in[:])
                Ms = dMp.tile([128, TOT_DEC // 128, D], BF16, tag="Ms")
                Md = dMp.tile([128, TOT_DEC // 128, D], BF16, tag="Md")
                res = doutp.tile([128, TOT_DEC // 128], F32)
                waited = set()
                coff = 0
                for g in gorder:
                    ks, kd = g // c.NGB, g % c.NGB
                    ncols = int(Tdec[g])
                    if ncols == 0:
                        continue
                    dep_s, dep_d = [], []
                    if ks not in waited:
                        dep_s = [ccst[(3, K)] for K in GB_CHUNKS[ks]]
                        waited.add(ks)
                    if kd not in waited:
                        dep_d = [ccst[(3, K)] for K in GB_CHUNKS[kd]]
                        waited.add(kd)
                    off16 = coff * 8
                    emit_gather(Ms[:, coff:coff + ncols, :], tslice(3, ks),
                                ds_sb[:, off16:off16 + ncols * 8], ncols * 128,
                                deps=dep_s)
                    emit_gather(Md[:, coff:coff + ncols, :], tslice(3, kd),
                                dd_sb[:, off16:off16 + ncols * 8], ncols * 128,
                                deps=dep_d)
                    for t in range(ncols):
                        col = coff + t
                        mm = dwp.tile([128, D], F32, tag="mm")
                        nc.vector.tensor_tensor(
                            mm[:], Ms[:, col, :], Md[:, col, :],
                            op=mybir.AluOpType.mult)
                        trash = dwp.tile([128, D], BF16, tag="tr")
                        nc.scalar.activation(
                            trash[:], mm[:], ACTF.Copy,
                            accum_out=res[:, col:col + 1])
                    coff += ncols
                nc.sync.dma_start(dots_out[:], res[:])

    nc.compile()
    return nc


def assemble_output(cfg, meta, results):
    c = cfg
    slot2j = meta["slot2j"]
    out = np.zeros(c.EL, dtype=np.float32)
    for core in range(len(results)):
        d = np.asarray(results[core]["dots"], dtype=np.float32)
        flat = d.T.reshape(-1)             # slot i -> d[i%128, i//128]
        s2j = slot2j[core]
        valid = s2j >= 0
        out[s2j[valid]] = flat[valid]
    return out


def run_pipeline(x, edge_index, edge_label_index, W1, b1, W2, b2,
                 cfg=None, trace=False, tmpdir=None):
    cfg = cfg or DEFAULT
    in_maps, meta = host_prep(cfg, x, edge_index, edge_label_index,
                              W1, b1, W2, b2)
    nc = build_program(cfg, meta)
    res = run_bass_kernel_spmd(nc, in_maps, list(range(cfg.NC)),
                               trace=trace, tmpdir=tmpdir)
    return assemble_output(cfg, meta, res.results), res


def kernel(x, edge_index, edge_label_index, W1, b1, W2, b2):
    out, _ = run_pipeline(x, edge_index, edge_label_index, W1, b1, W2, b2)
    return out


# revision 48
# speedup vs baseline: 1.0684x; 1.0684x over previous
"""Trainium2 Bass kernel for nn_LinkPredictor (2-layer GCN + edge-dot decode).

Strategy (8 NeuronCores, SPMD), v4:
  - Nodes sharded: core c owns rows [c*12544, (c+1)*12544) of the padded
    node table (N=100000 padded to 100352 = 8*98*128).
  - dinv folded into node features: table rows hold hs = dinv[n] * (prev @ W);
    output z = relu(dinv[v]*(agg + hs[v]) + b).
  - Node tables in DRAM are band-major contiguous: collective chunk K holds
    rows c*CH+j of each core's shard; 4 pipelined AllGathers per layer write
    slices of one tensor. Chunk sizes [14,32,32,20] windows: small first
    chunk so layer-1 gathers start early, small last chunk for a short
    decode tail.
  - Gathers use int16 indices relative to per-band bases; band == chunk.
    One dma_gather per (window-batch, gather-band), WB=4 windows per batch.
  - Aggregation: one-hot S built in WIDE batched DVE ops (one tensor_tensor
    is_equal per (batch, band) span using stride-0 broadcast APs) feeding
    PE matmul accumulation into PSUM. Self-loop = identity matmul; relu+dinv
    scale on ScalarE.
  - hs tiles resident in SBUF as wide [128, 12544] tiles; layer-2 output z
    aliases the layer-1 hs tile.
  - Next layer's first batches are prefetched during the current layer's
    tail through a shared M pool.
  - Decode: gathers z[s], z[d] by gather-band pair, one DVE
    tensor_tensor_reduce (mult+add) per tile.
"""
import contextlib
import math
import numpy as np
import ml_dtypes

import concourse.bass as bass
import concourse.tile as tile
from concourse import bacc, mybir
from concourse.bass_utils import run_bass_kernel_spmd
from concourse.tile_rust import add_dep_helper

F32 = mybir.dt.float32
BF16 = mybir.dt.bfloat16
I16 = mybir.dt.int16
BF = ml_dtypes.bfloat16
ACTF = mybir.ActivationFunctionType


class Cfg:
    def __init__(self, N=100000, E=1600000, EL=100000, D=128, ncores=8,
                 nw=98, wb=3, prefetch=3, ch_win=(8, 38, 38, 14),
                 band_split=2):
        self.N, self.E, self.EL, self.D, self.NC = N, E, EL, D, ncores
        self.NW = nw                      # windows (128 nodes each) per core
        self.SHARD = nw * 128             # nodes per core (padded)
        self.NP = self.SHARD * ncores     # padded node count
        assert self.NP >= N
        # collective chunks (rows per core), window-aligned; first chunk
        # small so the first AllGather (and layer-1 gathers) start early,
        # last chunk small so the decode tail is short.
        self.CH_WIN = list(ch_win)        # windows per chunk
        assert sum(self.CH_WIN) == nw
        self.NB = len(self.CH_WIN)
        self.CH_SIZES = [wn * 128 for wn in self.CH_WIN]
        self.CH_STARTS = np.cumsum([0] + self.CH_SIZES).tolist()
        self.TB_SIZES = [s * ncores for s in self.CH_SIZES]
        self.TB_STARTS = np.cumsum([0] + self.TB_SIZES).tolist()
        # 2 gather bands of <=65536 table rows each (signed int16 indices
        # relative to a mid-band base; hardware sign-extends). Band g covers
        # collective chunks GB_CHUNKS[g]; many small chunks keep each
        # AllGather short so layer boundaries don't stall on a big one.
        self.NGB = 2
        bs = self.BAND_SPLIT = band_split
        self.GB_CHUNKS = [list(range(bs)), list(range(bs, self.NB))]
        self.BAND_LO = [self.TB_STARTS[0], self.TB_STARTS[bs]]
        self.BAND_HI = [self.TB_STARTS[bs], self.TB_STARTS[self.NB]]
        for g in range(self.NGB):
            assert self.BAND_HI[g] - self.BAND_LO[g] <= 65536
        self.GBASE = [self.BAND_LO[g] + 32768 for g in range(self.NGB)]
        self.WB = wb                      # windows per gather/aggregate batch
        self.NBATCH = math.ceil(nw / wb)
        self.PREFETCH = prefetch          # next-layer batches gathered early


DEFAULT = Cfg()


def _wrap_idxs(idx):
    """[n] ints -> [128, n//16] int16 wrapped in 16 partitions, replicated 8x."""
    n = len(idx)
    assert n % 16 == 0
    w = np.asarray(idx, dtype=np.int16).reshape(n // 16, 16).T
    return np.ascontiguousarray(np.tile(w, (8, 1)))


def host_prep(cfg, x, edge_index, edge_label_index, W1, b1, W2, b2):
    """All host-side sharding/layout. Returns (per-core input maps, meta)."""
    c = cfg
    src = np.asarray(edge_index[0], dtype=np.int64)
    dst = np.asarray(edge_index[1], dtype=np.int64)
    deg = np.bincount(dst, minlength=c.N).astype(np.float64) + 1.0
    dinv = 1.0 / np.sqrt(deg)                      # [N]
    dinv_p = np.ones(c.NP, dtype=np.float64)
    dinv_p[:c.N] = dinv
    dinv_f = dinv_p.astype(np.float32)

    ch_starts = np.asarray(c.CH_STARTS[:-1])
    def chunk_of(r):
        return np.searchsorted(ch_starts, r, side="right") - 1

    def bmaj_of(n):
        """band-major table row of node id n."""
        cc, r = n // c.SHARD, n % c.SHARD
        K = chunk_of(r)
        return (np.asarray(c.TB_STARTS)[K] + cc * np.asarray(c.CH_SIZES)[K]
                + (r - ch_starts[K]))

    bmaj_src = bmaj_of(src)
    gband_src = (chunk_of(src % c.SHARD) >= c.BAND_SPLIT).astype(np.int64)
    core_of = dst // c.SHARD
    w_of = (dst % c.SHARD) // 128
    dloc = dst % 128

    key = (core_of * c.NW + w_of) * c.NGB + gband_src
    ngroups = c.NC * c.NW * c.NGB
    order = np.argsort(key, kind="stable")
    counts = np.bincount(key, minlength=ngroups).reshape(c.NC, c.NW, c.NGB)
    starts = np.zeros(ngroups + 1, dtype=np.int64)
    np.cumsum(np.bincount(key, minlength=ngroups), out=starts[1:])

    T = np.ceil(counts.max(axis=0) / 128).astype(np.int64)     # [NW, NGB]
    TOT_TILES = int(T.sum())
    span_tiles = np.zeros((c.NBATCH, c.NGB), dtype=np.int64)
    for b in range(c.NBATCH):
        wlo, whi = b * c.WB, min((b + 1) * c.WB, c.NW)
        for g in range(c.NGB):
            span_tiles[b, g] = T[wlo:whi, g].sum()
    TOT = TOT_TILES * 128

    idx_arr = np.zeros((c.NC, TOT), dtype=np.int64)
    dloc_arr = np.full((c.NC, TOT), -1.0, dtype=np.float32)
    for core in range(c.NC):
        pos = 0
        for b in range(c.NBATCH):
            wlo, whi = b * c.WB, min((b + 1) * c.WB, c.NW)
            for g in range(c.NGB):
                grp_start = pos
                for w in range(wlo, whi):
                    gk = (core * c.NW + w) * c.NGB + g
                    eids = order[starts[gk]:starts[gk + 1]]
                    n = len(eids)
                    idx_arr[core, pos:pos + n] = bmaj_src[eids] - c.GBASE[g]
                    dloc_arr[core, pos:pos + n] = dloc[eids]
                    grp_start = pos
                    pos += int(T[w, g]) * 128
                # Each (b, g) span is gathered as TWO half-gathers (to spread
                # across SWDGE queues). The ucode strips TRAILING negative
                # indices from each gather: the final slot of each half must
                # be >= 0. Swap within the window group containing that slot
                # (slots in a group share the same dst window, so any
                # permutation is safe).
                span_lo = pos - int(span_tiles[b, g]) * 128
                K = int(span_tiles[b, g])
                h1 = (K + 1) // 2
                ends = ([pos - 1] if K > 0 else []) + \
                       ([span_lo + h1 * 128 - 1] if 0 < h1 < K else [])
                for endslot in ends:
                    if idx_arr[core, endslot] >= 0:
                        continue
                    # find the (w,g) group containing endslot
                    gl = span_lo
                    for w in range(wlo, whi):
                        gh = gl + int(T[w, g]) * 128
                        if gl <= endslot < gh:
                            break
                        gl = gh
                    cand = np.nonzero(idx_arr[core, gl:gh] >= 0)[0] + gl
                    cand = [j for j in cand if j not in ends]
                    assert len(cand) > 0, "all-negative group"
                    j = cand[0]
                    for arr in (idx_arr, dloc_arr):
                        arr[core, j], arr[core, endslot] = \
                            arr[core, endslot], arr[core, j]
        assert pos == TOT
    assert idx_arr.min() >= -32768 and idx_arr.max() < 32768

    # decode: label edge j -> core j // ELC; groups by (gband(s), gband(d))
    assert c.EL % c.NC == 0
    ELC = c.EL // c.NC
    ls = np.asarray(edge_label_index[0], dtype=np.int64)
    ld = np.asarray(edge_label_index[1], dtype=np.int64)
    bs, bd = bmaj_of(ls), bmaj_of(ld)
    gs = (chunk_of(ls % c.SHARD) >= c.BAND_SPLIT).astype(np.int64)
    gd = (chunk_of(ld % c.SHARD) >= c.BAND_SPLIT).astype(np.int64)
    gdec = gs * c.NGB + gd
    NG_DEC = c.NGB * c.NGB
    cnt_dec = np.zeros((c.NC, NG_DEC), dtype=np.int64)
    for core in range(c.NC):
        cnt_dec[core] = np.bincount(gdec[core * ELC:(core + 1) * ELC],
                                    minlength=NG_DEC)
    Tdec = np.ceil(cnt_dec.max(axis=0) / 128).astype(np.int64)   # [NG_DEC]
    gorder = sorted(range(NG_DEC), key=lambda g: (max(g // c.NGB, g % c.NGB), g))
    TOT_DEC = int(Tdec.sum()) * 128
    idx_s = np.zeros((c.NC, TOT_DEC), dtype=np.int64)
    idx_d = np.zeros((c.NC, TOT_DEC), dtype=np.int64)
    slot2j = np.full((c.NC, TOT_DEC), -1, dtype=np.int64)
    for core in range(c.NC):
        jlo = core * ELC
        kk = gdec[jlo:jlo + ELC]
        o = np.argsort(kk, kind="stable")
        st = np.zeros(NG_DEC + 1, dtype=np.int64)
        np.cumsum(np.bincount(kk, minlength=NG_DEC), out=st[1:])
        pos = 0
        for g in gorder:
            js = o[st[g]:st[g + 1]] + jlo
            n = len(js)
            idx_s[core, pos:pos + n] = bs[js] - c.GBASE[g // c.NGB]
            idx_d[core, pos:pos + n] = bd[js] - c.GBASE[g % c.NGB]
            slot2j[core, pos:pos + n] = js
            p1 = pos + int(Tdec[g]) * 128
            # keep the final slot of each decode gather span non-negative
            # (the ucode strips trailing negative indices)
            if p1 > pos and (idx_s[core, p1 - 1] < 0 or
                             idx_d[core, p1 - 1] < 0):
                ok = np.nonzero((idx_s[core, pos:p1] >= 0) &
                                (idx_d[core, pos:p1] >= 0))[0]
                assert len(ok) > 0
                j = pos + ok[0]
                for a2 in (idx_s, idx_d, slot2j):
                    a2[core, j], a2[core, p1 - 1] = \
                        a2[core, p1 - 1], a2[core, j]
            pos = p1
        assert pos == TOT_DEC

    xp = np.zeros((c.NP, c.D), dtype=np.float32)
    xp[:c.N] = np.asarray(x, dtype=np.float32)
    use_b1 = bool(np.any(np.asarray(b1)))
    use_b2 = bool(np.any(np.asarray(b2)))
    assert not (use_b1 or use_b2), "bias path not wired in v10"

    # xs = dinv * x, in band-major table order: layer-1 gathers read this
    # host-provided table directly -> no P0 / no table-1 collectives.
    xs = xp * dinv_f[:, None]
    perm = bmaj_of(np.arange(c.NP))        # node id -> table row
    xs_tab = np.zeros((c.NP, c.D), dtype=np.float32)
    xs_tab[perm] = xs
    xs_tab = np.ascontiguousarray(xs_tab).astype(BF)

    in_maps = []
    for core in range(c.NC):
        sl = slice(core * c.SHARD, (core + 1) * c.SHARD)
        dsh = dinv_f[sl]
        m = {
            "xs_tab": xs_tab,
            "xs_own": np.ascontiguousarray(
                xs[sl].reshape(c.NW, 128, c.D).transpose(1, 0, 2)
                .reshape(128, c.NW * c.D)).astype(BF),
            "W1": np.asarray(W1, dtype=np.float32).astype(BF),
            "W2": np.asarray(W2, dtype=np.float32).astype(BF),
            "dinv": np.ascontiguousarray(dsh.reshape(c.NW, 128).T),
            "gidx": _wrap_idxs(idx_arr[core]),
            "dstloc": np.ascontiguousarray(
                dloc_arr[core].reshape(TOT_TILES, 128).T).astype(BF),
            "didx_s": _wrap_idxs(idx_s[core]),
            "didx_d": _wrap_idxs(idx_d[core]),
        }
        in_maps.append(m)
    meta = dict(T=T, span_tiles=span_tiles, TOT=TOT, TOT_TILES=TOT_TILES,
                Tdec=Tdec, gorder=gorder, TOT_DEC=TOT_DEC, slot2j=slot2j,
                use_b1=use_b1, use_b2=use_b2)
    return in_maps, meta


def build_program(cfg, meta, num_cores=None):
    c = cfg
    NCores = num_cores or c.NC
    T, span_tiles = meta["T"], meta["span_tiles"]
    TOT, TOT_TILES = meta["TOT"], meta["TOT_TILES"]
    Tdec, gorder, TOT_DEC = meta["Tdec"], meta["gorder"], meta["TOT_DEC"]
    use_b = {1: meta["use_b1"], 2: meta["use_b2"]}
    D = c.D
    TBMAX = int(span_tiles.sum(axis=1).max())
    SPANMAX = int(span_tiles.max())
    GB_CHUNKS = c.GB_CHUNKS

    nc = bacc.Bacc("TRN2", target_bir_lowering=False, debug=False,
                   num_devices=NCores, num_swdge_queues=4)
    NQ = 4

    assert not (use_b[1] or use_b[2])
    xs_tab_in = nc.dram_tensor("xs_tab", [c.TB_STARTS[-1], D], BF16,
                               kind="ExternalInput")
    xs_own_in = nc.dram_tensor("xs_own", [128, c.SHARD], BF16,
                               kind="ExternalInput")
    W1_in = nc.dram_tensor("W1", [D, D], BF16, kind="ExternalInput")
    W2_in = nc.dram_tensor("W2", [D, D], BF16, kind="ExternalInput")
    dinv_in = nc.dram_tensor("dinv", [128, c.NW], F32, kind="ExternalInput")
    gidx_in = nc.dram_tensor("gidx", [128, TOT // 16], I16, kind="ExternalInput")
    dstloc_in = nc.dram_tensor("dstloc", [128, TOT_TILES], BF16, kind="ExternalInput")
    didx_s_in = nc.dram_tensor("didx_s", [128, TOT_DEC // 16], I16, kind="ExternalInput")
    didx_d_in = nc.dram_tensor("didx_d", [128, TOT_DEC // 16], I16, kind="ExternalInput")
    dots_out = nc.dram_tensor("dots", [128, TOT_DEC // 128], F32, kind="ExternalOutput")

    shard_b = {l: [nc.dram_tensor(f"shard{l}_{k}", [c.CH_SIZES[k], D], BF16)
                   for k in range(c.NB)] for l in (2, 3)}
    table = {l: nc.dram_tensor(f"table{l}", [c.TB_STARTS[-1], D], BF16,
                               addr_space="Shared") for l in (2, 3)}

    def tslice(l, g):
        """Gather source AP for band g: base at GBASE[g] (mid-band); signed
        int16 indices reach the whole band [BAND_LO, BAND_HI). Layer 1
        gathers the host-provided xs table (no collective needed)."""
        src = xs_tab_in if l == 1 else table[l]
        return src[c.GBASE[g]:c.BAND_HI[g], :]

    iota_dram = nc.inline_tensor(
        np.tile(np.arange(128, dtype=np.float32), (128, 1)).astype(BF), "iota_c")
    ident_dram = nc.inline_tensor(np.eye(128, dtype=np.float32).astype(BF), "ident_c")

    core_ids = list(range(NCores))
    gst = {"count": 0, "prev": None}
    ccst = {}                            # (l, K) -> collective instruction

    def emit_gather(out_ap, in_ap, idx_ap, n_idx, deps=()):
        q = gst["count"] % NQ
        inst = nc.gpsimd.dma_gather(out_ap, in_ap, idx_ap, n_idx, n_idx, D,
                                    queue_num=q, single_packet=False)
        for dcc in deps:
            add_dep_helper(inst.ins, dcc.ins, sync=True,
                           reason="gather after collective")
        gst["count"] += 1
        return inst

    def emit_collective(l, K):
        cc = nc.gpsimd.collective_compute(
            "AllGather", mybir.AluOpType.bypass,
            replica_groups=[core_ids],
            ins=[shard_b[l][K][:]],
            outs=[table[l][c.TB_STARTS[K]:c.TB_STARTS[K + 1], :]],
        )
        ccst[(l, K)] = cc
        return cc

    def write_window(l, w, src_ap, done_k):
        """DMA window w rows into its chunk shard; fire collectives when a
        chunk completes (chunks are window-aligned)."""
        lo = w * 128
        K = 0
        while c.CH_STARTS[K + 1] <= lo:
            K += 1
        off = lo - c.CH_STARTS[K]
        nc.sync.dma_start(shard_b[l][K][off:off + 128, :], src_ap)
        while len(done_k) < c.NB and \
                (w + 1) * 128 >= c.CH_STARTS[len(done_k) + 1]:
            emit_collective(l, len(done_k))
            done_k.append(len(done_k))

    with tile.TileContext(nc) as tc:
        with contextlib.ExitStack() as es:
            const = es.enter_context(tc.tile_pool(name="const", bufs=1))
            meta_p = es.enter_context(tc.tile_pool(name="meta", bufs=1))

            w1_sb = const.tile([D, D], BF16); nc.sync.dma_start(w1_sb[:], W1_in[:])
            w2_sb = const.tile([D, D], BF16); nc.sync.dma_start(w2_sb[:], W2_in[:])
            dinv_sb = const.tile([128, c.NW], F32)
            nc.sync.dma_start(dinv_sb[:], dinv_in[:])
            iota_sb = const.tile([128, 128], BF16)
            nc.sync.dma_start(iota_sb[:], iota_dram[:])
            ident_sb = const.tile([128, 128], BF16)
            nc.sync.dma_start(ident_sb[:], ident_dram[:])
            # gidx first: the first gathers need it; dstloc/xs_own follow on
            # other queues
            gidx_sb = meta_p.tile([128, TOT // 16], I16)
            nc.scalar.dma_start(gidx_sb[:], gidx_in[:])
            dstloc_sb = meta_p.tile([128, TOT_TILES], BF16)
            nc.scalar.dma_start(dstloc_sb[:], dstloc_in[:])
            xs_own_sb = meta_p.tile([128, c.SHARD], BF16)
            nc.sync.dma_start(xs_own_sb[:], xs_own_in[:])

            span_base = {}
            tcol0 = 0
            for b in range(c.NBATCH):
                m0 = 0
                for g in range(c.NGB):
                    span_base[(b, g)] = (tcol0, m0)
                    tcol0 += int(span_tiles[b, g])
                    m0 += int(span_tiles[b, g])

            pre = {}        # (lid, b) -> (dict g -> Mt, set of emitted gbands)
            cc_waited = {1: set(), 2: set(), 3: set()}
            SPANG = [int(span_tiles[:, g].max()) for g in range(c.NGB)]

            def batch_gathers(Mp, lid, b, only_avail=False):
                """Emit (remaining) gathers for batch b of layer lid. Per-band
                M pools: band A (early-available) pipelines deeply without
                waiting for band B's collectives."""
                Mts, done = pre.get((lid, b), ({}, set()))
                for g in range(c.NGB):
                    if g in done or int(span_tiles[b, g]) == 0:
                        continue
                    if lid > 1 and only_avail and any((lid, K) not in ccst
                                                      for K in GB_CHUNKS[g]):
                        continue
                    ntiles = int(span_tiles[b, g])
                    tb, _ = span_base[(b, g)]
                    deps = []
                    if lid > 1 and g not in cc_waited[lid]:
                        deps = [ccst[(lid, K)] for K in GB_CHUNKS[g]]
                        cc_waited[lid].add(g)
                    Mt = Mp.tile([128, SPANG[g], 128], BF16, tag=f"M{g}",
                                 bufs=(8 if g == 0 else 3))
                    # two half-gathers per span: spreads each batch across
                    # all 4 SWDGE queues (desc-gen parallelizes per queue)
                    h1 = (ntiles + 1) // 2
                    for lo, hi in ((0, h1), (h1, ntiles)):
                        if hi > lo:
                            emit_gather(
                                Mt[:, lo:hi, :], tslice(lid, g),
                                gidx_sb[:, (tb + lo) * 8:(tb + hi) * 8],
                                (hi - lo) * 128, deps=deps)
                    Mts[g] = Mt
                    done.add(g)
                pre[(lid, b)] = (Mts, done)
                return Mts

            def build_spans(Sp, b):
                """One wide DVE is_equal per (batch, band) span: all one-hot
                S tiles of the span in a single instruction."""
                Sw = {}
                for g in range(c.NGB):
                    K = int(span_tiles[b, g])
                    if K == 0:
                        continue
                    tb, mb = span_base[(b, g)]
                    St = Sp.tile([128, SPANG[g], 128], BF16, tag=f"S{g}",
                                 bufs=3)
                    nc.vector.tensor_tensor(
                        St[:, :K, :],
                        iota_sb[:].unsqueeze(1).broadcast_to([128, K, 128]),
                        dstloc_sb[:, tb:tb + K].unsqueeze(2)
                            .broadcast_to([128, K, 128]),
                        op=mybir.AluOpType.is_equal)
                    Sw[g] = (St, mb)
                return Sw

            def layer(Mp, lid, hall, hall_next, next_lid):
                """lid==1: aggregate raw xs, then apply W1 (transpose+matmul)
                and W2 per window to produce hs2. lid==2: aggregate hs2,
                relu+scale to produce z."""
                waited_done = []
                with tc.tile_pool(name=f"S{lid}", bufs=6) as Sp, \
                     tc.tile_pool(name=f"ag{lid}", bufs=4, space="PSUM") as agp, \
                     tc.tile_pool(name=f"tp{lid}", bufs=1, space="PSUM") as tpp, \
                     tc.tile_pool(name=f"ep{lid}", bufs=4) as epp:
                    for b in range(c.NBATCH):
                        wlo, whi = b * c.WB, min((b + 1) * c.WB, c.NW)
                        Mts = batch_gathers(Mp, lid, b)
                        Sw = build_spans(Sp, b)
                        for w in range(wlo, whi):
                            ps = agp.tile([128, D], F32, tag="agg")
                            nmm = int(T[w].sum())
                            hsl = hall[:, w * 128:w * 128 + D]
                            nc.tensor.matmul(ps[:], lhsT=ident_sb[:],
                                             rhs=hsl,
                                             start=True, stop=(nmm == 0))
                            mi = 0
                            for g in range(c.NGB):
                                if int(T[w, g]) == 0:
                                    continue
                                St, _ = Sw[g]
                                Mt = Mts[g]
                                off = int(T[wlo:w, g].sum())
                                for t in range(int(T[w, g])):
                                    k = off + t
                                    mi += 1
                                    nc.tensor.matmul(
                                        ps[:], lhsT=St[:, k, :],
                                        rhs=Mt[:, k, :],
                                        start=False,
                                        stop=(mi == nmm))
                            zo = None
                            if lid == 1:
                                # a1 = dinv*(agg+self)  [pre-W1 aggregate]
                                a1 = epp.tile([128, D], BF16, tag="a1")
                                nc.scalar.activation(
                                    a1[:], ps[:], ACTF.Copy,
                                    scale=dinv_sb[:, w:w + 1])
                                t1ps = tpp.tile([128, D], BF16, tag="t1")
                                nc.tensor.transpose(t1ps[:], a1[:], ident_sb[:])
                                a1T = epp.tile([128, D], BF16, tag="a1T")
                                nc.scalar.activation(a1T[:], t1ps[:], ACTF.Copy)
                                yps = tpp.tile([128, D], F32, tag="y")
                                nc.tensor.matmul(yps[:], lhsT=a1T[:],
                                                 rhs=w1_sb[:],
                                                 start=True, stop=True)
                                z = epp.tile([128, D], BF16, tag="z")
                                nc.scalar.activation(z[:], yps[:], ACTF.Relu)
                                t2ps = tpp.tile([128, D], BF16, tag="t2")
                                nc.tensor.transpose(t2ps[:], z[:], ident_sb[:])
                                zT = epp.tile([128, D], BF16, tag="zT")
                                nc.scalar.activation(zT[:], t2ps[:], ACTF.Copy)
                                h2ps = tpp.tile([128, D], F32, tag="h2")
                                nc.tensor.matmul(h2ps[:], lhsT=zT[:],
                                                 rhs=w2_sb[:],
                                                 start=True, stop=True)
                                nc.scalar.activation(
                                    hall_next[:, w * 128:w * 128 + D], h2ps[:],
                                    ACTF.Copy, scale=dinv_sb[:, w:w + 1])
                            else:
                                # layer-2 output only stages toward the shard
                                # DMA; use a small rolling buffer
                                zo = epp.tile([128, D], BF16, tag="zo")
                                nc.scalar.activation(
                                    zo[:], ps[:],
                                    ACTF.Relu, scale=dinv_sb[:, w:w + 1])
                            out_ap = (hall_next[:, w * 128:w * 128 + D]
                                      if lid == 1 else zo[:])
                            write_window(next_lid, w, out_ap, waited_done)
                        if b == c.NBATCH - 1 - c.PREFETCH and next_lid == 2:
                            for pb in range(min(c.PREFETCH + 1, c.NBATCH)):
                                batch_gathers(Mp, 2, pb, only_avail=True)

            with tc.tile_pool(name="hs", bufs=1) as hsp, \
                 tc.tile_pool(name="Mpool", bufs=1) as Mp:
                hs2_all = hsp.tile([128, c.SHARD], BF16, tag="hs2")
                layer(Mp, 1, xs_own_sb, hs2_all, 2)
                layer(Mp, 2, hs2_all, None, 3)

            # decode
            with tc.tile_pool(name="didx", bufs=1) as didxp, \
                 tc.tile_pool(name="dM", bufs=1) as dMp, \
                 tc.tile_pool(name="dw", bufs=6) as dwp, \
                 tc.tile_pool(name="dout", bufs=1) as doutp:
                ds_sb = didxp.tile([128, TOT_DEC // 16], I16)
                nc.scalar.dma_start(ds_sb[:], didx_s_in[:])
                dd_sb = didxp.tile([128, TOT_DEC // 16], I16)
                nc.scalar.dma_start(dd_sb[:], didx_d_in[:])
                Ms = dMp.tile([128, TOT_DEC // 128, D], BF16, tag="Ms")
                Md = dMp.tile([128, TOT_DEC // 128, D], BF16, tag="Md")
                res = doutp.tile([128, TOT_DEC // 128], F32)
                waited = set()
                coff = 0
                for g in gorder:
                    ks, kd = g // c.NGB, g % c.NGB
                    ncols = int(Tdec[g])
                    if ncols == 0:
                        continue
                    dep_s, dep_d = [], []
                    if ks not in waited:
                        dep_s = [ccst[(3, K)] for K in GB_CHUNKS[ks]]
                        waited.add(ks)
                    if kd not in waited:
                        dep_d = [ccst[(3, K)] for K in GB_CHUNKS[kd]]
                        waited.add(kd)
                    off16 = coff * 8
                    emit_gather(Ms[:, coff:coff + ncols, :], tslice(3, ks),
                                ds_sb[:, off16:off16 + ncols * 8], ncols * 128,
                                deps=dep_s)
                    emit_gather(Md[:, coff:coff + ncols, :], tslice(3, kd),
                                dd_sb[:, off16:off16 + ncols * 8], ncols * 128,
                                deps=dep_d)
                    for t in range(ncols):
                        col = coff + t
                        mm = dwp.tile([128, D], F32, tag="mm")
                        nc.vector.tensor_tensor(
                            mm[:], Ms[:, col, :], Md[:, col, :],
                            op=mybir.AluOpType.mult)
                        trash = dwp.tile([128, D], BF16, tag="tr")
                        nc.scalar.activation(
                            trash[:], mm[:], ACTF.Copy,
                            accum_out=res[:, col:col + 1])
                    coff += ncols
                nc.sync.dma_start(dots_out[:], res[:])

    nc.compile()
    return nc


def assemble_output(cfg, meta, results):
    c = cfg
    slot2j = meta["slot2j"]
    out = np.zeros(c.EL, dtype=np.float32)
    for core in range(len(results)):
        d = np.asarray(results[core]["dots"], dtype=np.float32)
        flat = d.T.reshape(-1)             # slot i -> d[i%128, i//128]
        s2j = slot2j[core]
        valid = s2j >= 0
        out[s2j[valid]] = flat[valid]
    return out


def run_pipeline(x, edge_index, edge_label_index, W1, b1, W2, b2,
                 cfg=None, trace=False, tmpdir=None):
    cfg = cfg or DEFAULT
    in_maps, meta = host_prep(cfg, x, edge_index, edge_label_index,
                              W1, b1, W2, b2)
    nc = build_program(cfg, meta)
    res = run_bass_kernel_spmd(nc, in_maps, list(range(cfg.NC)),
                               trace=trace, tmpdir=tmpdir)
    return assemble_output(cfg, meta, res.results), res


def kernel(x, edge_index, edge_label_index, W1, b1, W2, b2):
    out, _ = run_pipeline(x, edge_index, edge_label_index, W1, b1, W2, b2)
    return out


# revision 52
# speedup vs baseline: 1.0826x; 1.0133x over previous
"""Trainium2 Bass kernel for nn_LinkPredictor (2-layer GCN + edge-dot decode).

Strategy (8 NeuronCores, SPMD), v4:
  - Nodes sharded: core c owns rows [c*12544, (c+1)*12544) of the padded
    node table (N=100000 padded to 100352 = 8*98*128).
  - dinv folded into node features: table rows hold hs = dinv[n] * (prev @ W);
    output z = relu(dinv[v]*(agg + hs[v]) + b).
  - Node tables in DRAM are band-major contiguous: collective chunk K holds
    rows c*CH+j of each core's shard; 4 pipelined AllGathers per layer write
    slices of one tensor. Chunk sizes [14,32,32,20] windows: small first
    chunk so layer-1 gathers start early, small last chunk for a short
    decode tail.
  - Gathers use int16 indices relative to per-band bases; band == chunk.
    One dma_gather per (window-batch, gather-band), WB=4 windows per batch.
  - Aggregation: one-hot S built in WIDE batched DVE ops (one tensor_tensor
    is_equal per (batch, band) span using stride-0 broadcast APs) feeding
    PE matmul accumulation into PSUM. Self-loop = identity matmul; relu+dinv
    scale on ScalarE.
  - hs tiles resident in SBUF as wide [128, 12544] tiles; layer-2 output z
    aliases the layer-1 hs tile.
  - Next layer's first batches are prefetched during the current layer's
    tail through a shared M pool.
  - Decode: gathers z[s], z[d] by gather-band pair, one DVE
    tensor_tensor_reduce (mult+add) per tile.
"""
import contextlib
import math
import numpy as np
import ml_dtypes

import concourse.bass as bass
import concourse.tile as tile
from concourse import bacc, mybir
from concourse.bass_utils import run_bass_kernel_spmd
from concourse.tile_rust import add_dep_helper

F32 = mybir.dt.float32
BF16 = mybir.dt.bfloat16
I16 = mybir.dt.int16
BF = ml_dtypes.bfloat16
ACTF = mybir.ActivationFunctionType


class Cfg:
    def __init__(self, N=100000, E=1600000, EL=100000, D=128, ncores=8,
                 nw=98, wb=3, prefetch=3, ch_win=(8, 31, 31, 28),
                 band_split=2):
        self.N, self.E, self.EL, self.D, self.NC = N, E, EL, D, ncores
        self.NW = nw                      # windows (128 nodes each) per core
        self.SHARD = nw * 128             # nodes per core (padded)
        self.NP = self.SHARD * ncores     # padded node count
        assert self.NP >= N
        # collective chunks (rows per core), window-aligned; first chunk
        # small so the first AllGather (and layer-1 gathers) start early,
        # last chunk small so the decode tail is short.
        self.CH_WIN = list(ch_win)        # windows per chunk
        assert sum(self.CH_WIN) == nw
        self.NB = len(self.CH_WIN)
        self.CH_SIZES = [wn * 128 for wn in self.CH_WIN]
        self.CH_STARTS = np.cumsum([0] + self.CH_SIZES).tolist()
        self.TB_SIZES = [s * ncores for s in self.CH_SIZES]
        self.TB_STARTS = np.cumsum([0] + self.TB_SIZES).tolist()
        # 2 gather bands of <=65536 table rows each (signed int16 indices
        # relative to a mid-band base; hardware sign-extends). Band g covers
        # collective chunks GB_CHUNKS[g]; many small chunks keep each
        # AllGather short so layer boundaries don't stall on a big one.
        self.NGB = 2
        bs = self.BAND_SPLIT = band_split
        self.GB_CHUNKS = [list(range(bs)), list(range(bs, self.NB))]
        self.BAND_LO = [self.TB_STARTS[0], self.TB_STARTS[bs]]
        self.BAND_HI = [self.TB_STARTS[bs], self.TB_STARTS[self.NB]]
        for g in range(self.NGB):
            assert self.BAND_HI[g] - self.BAND_LO[g] <= 65536
        self.GBASE = [self.BAND_LO[g] + 32768 for g in range(self.NGB)]
        self.WB = wb                      # windows per gather/aggregate batch
        self.NBATCH = math.ceil(nw / wb)
        self.PREFETCH = prefetch          # next-layer batches gathered early


DEFAULT = Cfg()


def _wrap_idxs(idx):
    """[n] ints -> [128, n//16] int16 wrapped in 16 partitions, replicated 8x."""
    n = len(idx)
    assert n % 16 == 0
    w = np.asarray(idx, dtype=np.int16).reshape(n // 16, 16).T
    return np.ascontiguousarray(np.tile(w, (8, 1)))


def host_prep(cfg, x, edge_index, edge_label_index, W1, b1, W2, b2):
    """All host-side sharding/layout. Returns (per-core input maps, meta)."""
    c = cfg
    src = np.asarray(edge_index[0], dtype=np.int64)
    dst = np.asarray(edge_index[1], dtype=np.int64)
    deg = np.bincount(dst, minlength=c.N).astype(np.float64) + 1.0
    dinv = 1.0 / np.sqrt(deg)                      # [N]
    dinv_p = np.ones(c.NP, dtype=np.float64)
    dinv_p[:c.N] = dinv
    dinv_f = dinv_p.astype(np.float32)

    ch_starts = np.asarray(c.CH_STARTS[:-1])
    def chunk_of(r):
        return np.searchsorted(ch_starts, r, side="right") - 1

    def bmaj_of(n):
        """band-major table row of node id n."""
        cc, r = n // c.SHARD, n % c.SHARD
        K = chunk_of(r)
        return (np.asarray(c.TB_STARTS)[K] + cc * np.asarray(c.CH_SIZES)[K]
                + (r - ch_starts[K]))

    bmaj_src = bmaj_of(src)
    gband_src = (chunk_of(src % c.SHARD) >= c.BAND_SPLIT).astype(np.int64)
    core_of = dst // c.SHARD
    w_of = (dst % c.SHARD) // 128
    dloc = dst % 128

    key = (core_of * c.NW + w_of) * c.NGB + gband_src
    ngroups = c.NC * c.NW * c.NGB
    order = np.argsort(key, kind="stable")
    counts = np.bincount(key, minlength=ngroups).reshape(c.NC, c.NW, c.NGB)
    starts = np.zeros(ngroups + 1, dtype=np.int64)
    np.cumsum(np.bincount(key, minlength=ngroups), out=starts[1:])

    T = np.ceil(counts.max(axis=0) / 128).astype(np.int64)     # [NW, NGB]
    TOT_TILES = int(T.sum())
    span_tiles = np.zeros((c.NBATCH, c.NGB), dtype=np.int64)
    for b in range(c.NBATCH):
        wlo, whi = b * c.WB, min((b + 1) * c.WB, c.NW)
        for g in range(c.NGB):
            span_tiles[b, g] = T[wlo:whi, g].sum()
    TOT = TOT_TILES * 128

    idx_arr = np.zeros((c.NC, TOT), dtype=np.int64)
    dloc_arr = np.full((c.NC, TOT), -1.0, dtype=np.float32)
    for core in range(c.NC):
        pos = 0
        for b in range(c.NBATCH):
            wlo, whi = b * c.WB, min((b + 1) * c.WB, c.NW)
            for g in range(c.NGB):
                grp_start = pos
                for w in range(wlo, whi):
                    gk = (core * c.NW + w) * c.NGB + g
                    eids = order[starts[gk]:starts[gk + 1]]
                    n = len(eids)
                    idx_arr[core, pos:pos + n] = bmaj_src[eids] - c.GBASE[g]
                    dloc_arr[core, pos:pos + n] = dloc[eids]
                    grp_start = pos
                    pos += int(T[w, g]) * 128
                # Each (b, g) span is gathered as TWO half-gathers (to spread
                # across SWDGE queues). The ucode strips TRAILING negative
                # indices from each gather: the final slot of each half must
                # be >= 0. Swap within the window group containing that slot
                # (slots in a group share the same dst window, so any
                # permutation is safe).
                span_lo = pos - int(span_tiles[b, g]) * 128
                K = int(span_tiles[b, g])
                h1 = (K + 1) // 2
                ends = ([pos - 1] if K > 0 else []) + \
                       ([span_lo + h1 * 128 - 1] if 0 < h1 < K else [])
                for endslot in ends:
                    if idx_arr[core, endslot] >= 0:
                        continue
                    # find the (w,g) group containing endslot
                    gl = span_lo
                    for w in range(wlo, whi):
                        gh = gl + int(T[w, g]) * 128
                        if gl <= endslot < gh:
                            break
                        gl = gh
                    cand = np.nonzero(idx_arr[core, gl:gh] >= 0)[0] + gl
                    cand = [j for j in cand if j not in ends]
                    assert len(cand) > 0, "all-negative group"
                    j = cand[0]
                    for arr in (idx_arr, dloc_arr):
                        arr[core, j], arr[core, endslot] = \
                            arr[core, endslot], arr[core, j]
        assert pos == TOT
    assert idx_arr.min() >= -32768 and idx_arr.max() < 32768

    # decode: label edge j -> core j // ELC; groups by (gband(s), gband(d))
    assert c.EL % c.NC == 0
    ELC = c.EL // c.NC
    ls = np.asarray(edge_label_index[0], dtype=np.int64)
    ld = np.asarray(edge_label_index[1], dtype=np.int64)
    bs, bd = bmaj_of(ls), bmaj_of(ld)
    gs = (chunk_of(ls % c.SHARD) >= c.BAND_SPLIT).astype(np.int64)
    gd = (chunk_of(ld % c.SHARD) >= c.BAND_SPLIT).astype(np.int64)
    gdec = gs * c.NGB + gd
    NG_DEC = c.NGB * c.NGB
    cnt_dec = np.zeros((c.NC, NG_DEC), dtype=np.int64)
    for core in range(c.NC):
        cnt_dec[core] = np.bincount(gdec[core * ELC:(core + 1) * ELC],
                                    minlength=NG_DEC)
    Tdec = np.ceil(cnt_dec.max(axis=0) / 128).astype(np.int64)   # [NG_DEC]
    gorder = sorted(range(NG_DEC), key=lambda g: (max(g // c.NGB, g % c.NGB), g))
    TOT_DEC = int(Tdec.sum()) * 128
    idx_s = np.zeros((c.NC, TOT_DEC), dtype=np.int64)
    idx_d = np.zeros((c.NC, TOT_DEC), dtype=np.int64)
    slot2j = np.full((c.NC, TOT_DEC), -1, dtype=np.int64)
    for core in range(c.NC):
        jlo = core * ELC
        kk = gdec[jlo:jlo + ELC]
        o = np.argsort(kk, kind="stable")
        st = np.zeros(NG_DEC + 1, dtype=np.int64)
        np.cumsum(np.bincount(kk, minlength=NG_DEC), out=st[1:])
        pos = 0
        for g in gorder:
            js = o[st[g]:st[g + 1]] + jlo
            n = len(js)
            idx_s[core, pos:pos + n] = bs[js] - c.GBASE[g // c.NGB]
            idx_d[core, pos:pos + n] = bd[js] - c.GBASE[g % c.NGB]
            slot2j[core, pos:pos + n] = js
            p1 = pos + int(Tdec[g]) * 128
            # keep the final slot of each decode half-gather non-negative
            # (the ucode strips trailing negative indices); decode slots are
            # freely permutable within a group (slot2j carries the mapping)
            Kd = int(Tdec[g])
            h1 = (Kd + 1) // 2
            ends = ([p1 - 1] if Kd else []) + \
                   ([pos + h1 * 128 - 1] if 0 < h1 < Kd else [])
            for endslot in ends:
                if idx_s[core, endslot] >= 0 and idx_d[core, endslot] >= 0:
                    continue
                ok = np.nonzero((idx_s[core, pos:p1] >= 0) &
                                (idx_d[core, pos:p1] >= 0))[0] + pos
                ok = [j for j in ok if j not in ends]
                assert len(ok) > 0
                j = ok[0]
                for a2 in (idx_s, idx_d, slot2j):
                    a2[core, j], a2[core, endslot] = \
                        a2[core, endslot], a2[core, j]
            pos = p1
        assert pos == TOT_DEC

    xp = np.zeros((c.NP, c.D), dtype=np.float32)
    xp[:c.N] = np.asarray(x, dtype=np.float32)
    use_b1 = bool(np.any(np.asarray(b1)))
    use_b2 = bool(np.any(np.asarray(b2)))
    assert not (use_b1 or use_b2), "bias path not wired in v10"

    # xs = dinv * x, in band-major table order: layer-1 gathers read this
    # host-provided table directly -> no P0 / no table-1 collectives.
    xs = xp * dinv_f[:, None]
    perm = bmaj_of(np.arange(c.NP))        # node id -> table row
    xs_tab = np.zeros((c.NP, c.D), dtype=np.float32)
    xs_tab[perm] = xs
    xs_tab = np.ascontiguousarray(xs_tab).astype(BF)

    in_maps = []
    for core in range(c.NC):
        sl = slice(core * c.SHARD, (core + 1) * c.SHARD)
        dsh = dinv_f[sl]
        m = {
            "xs_tab": xs_tab,
            "xs_own": np.ascontiguousarray(
                xs[sl].reshape(c.NW, 128, c.D).transpose(1, 0, 2)
                .reshape(128, c.NW * c.D)).astype(BF),
            "W1": np.asarray(W1, dtype=np.float32).astype(BF),
            "W2": np.asarray(W2, dtype=np.float32).astype(BF),
            "dinv": np.ascontiguousarray(dsh.reshape(c.NW, 128).T),
            "gidx": _wrap_idxs(idx_arr[core]),
            "dstloc": np.ascontiguousarray(
                dloc_arr[core].reshape(TOT_TILES, 128).T).astype(BF),
            "didx_s": _wrap_idxs(idx_s[core]),
            "didx_d": _wrap_idxs(idx_d[core]),
        }
        in_maps.append(m)
    meta = dict(T=T, span_tiles=span_tiles, TOT=TOT, TOT_TILES=TOT_TILES,
                Tdec=Tdec, gorder=gorder, TOT_DEC=TOT_DEC, slot2j=slot2j,
                use_b1=use_b1, use_b2=use_b2)
    return in_maps, meta


def build_program(cfg, meta, num_cores=None):
    c = cfg
    NCores = num_cores or c.NC
    T, span_tiles = meta["T"], meta["span_tiles"]
    TOT, TOT_TILES = meta["TOT"], meta["TOT_TILES"]
    Tdec, gorder, TOT_DEC = meta["Tdec"], meta["gorder"], meta["TOT_DEC"]
    use_b = {1: meta["use_b1"], 2: meta["use_b2"]}
    D = c.D
    TBMAX = int(span_tiles.sum(axis=1).max())
    SPANMAX = int(span_tiles.max())
    DEC_SPANMAX = int(Tdec.max())
    GB_CHUNKS = c.GB_CHUNKS

    nc = bacc.Bacc("TRN2", target_bir_lowering=False, debug=False,
                   num_devices=NCores, num_swdge_queues=4)
    NQ = 4

    assert not (use_b[1] or use_b[2])
    xs_tab_in = nc.dram_tensor("xs_tab", [c.TB_STARTS[-1], D], BF16,
                               kind="ExternalInput")
    xs_own_in = nc.dram_tensor("xs_own", [128, c.SHARD], BF16,
                               kind="ExternalInput")
    W1_in = nc.dram_tensor("W1", [D, D], BF16, kind="ExternalInput")
    W2_in = nc.dram_tensor("W2", [D, D], BF16, kind="ExternalInput")
    dinv_in = nc.dram_tensor("dinv", [128, c.NW], F32, kind="ExternalInput")
    gidx_in = nc.dram_tensor("gidx", [128, TOT // 16], I16, kind="ExternalInput")
    dstloc_in = nc.dram_tensor("dstloc", [128, TOT_TILES], BF16, kind="ExternalInput")
    didx_s_in = nc.dram_tensor("didx_s", [128, TOT_DEC // 16], I16, kind="ExternalInput")
    didx_d_in = nc.dram_tensor("didx_d", [128, TOT_DEC // 16], I16, kind="ExternalInput")
    dots_out = nc.dram_tensor("dots", [128, TOT_DEC // 128], F32, kind="ExternalOutput")

    shard_b = {l: [nc.dram_tensor(f"shard{l}_{k}", [c.CH_SIZES[k], D], BF16)
                   for k in range(c.NB)] for l in (2, 3)}
    table = {l: nc.dram_tensor(f"table{l}", [c.TB_STARTS[-1], D], BF16,
                               addr_space="Shared") for l in (2, 3)}

    def tslice(l, g):
        """Gather source AP for band g: base at GBASE[g] (mid-band); signed
        int16 indices reach the whole band [BAND_LO, BAND_HI). Layer 1
        gathers the host-provided xs table (no collective needed)."""
        src = xs_tab_in if l == 1 else table[l]
        return src[c.GBASE[g]:c.BAND_HI[g], :]

    iota_dram = nc.inline_tensor(
        np.tile(np.arange(128, dtype=np.float32), (128, 1)).astype(BF), "iota_c")
    ident_dram = nc.inline_tensor(np.eye(128, dtype=np.float32).astype(BF), "ident_c")

    core_ids = list(range(NCores))
    gst = {"count": 0, "prev": None}
    ccst = {}                            # (l, K) -> collective instruction

    def emit_gather(out_ap, in_ap, idx_ap, n_idx, deps=()):
        q = gst["count"] % NQ
        inst = nc.gpsimd.dma_gather(out_ap, in_ap, idx_ap, n_idx, n_idx, D,
                                    queue_num=q, single_packet=False)
        for dcc in deps:
            add_dep_helper(inst.ins, dcc.ins, sync=True,
                           reason="gather after collective")
        gst["count"] += 1
        return inst

    def emit_collective(l, K):
        cc = nc.gpsimd.collective_compute(
            "AllGather", mybir.AluOpType.bypass,
            replica_groups=[core_ids],
            ins=[shard_b[l][K][:]],
            outs=[table[l][c.TB_STARTS[K]:c.TB_STARTS[K + 1], :]],
        )
        ccst[(l, K)] = cc
        return cc

    def write_window(l, w, src_ap, done_k):
        """DMA window w rows into its chunk shard; fire collectives when a
        chunk completes (chunks are window-aligned)."""
        lo = w * 128
        K = 0
        while c.CH_STARTS[K + 1] <= lo:
            K += 1
        off = lo - c.CH_STARTS[K]
        nc.sync.dma_start(shard_b[l][K][off:off + 128, :], src_ap)
        while len(done_k) < c.NB and \
                (w + 1) * 128 >= c.CH_STARTS[len(done_k) + 1]:
            emit_collective(l, len(done_k))
            done_k.append(len(done_k))

    with tile.TileContext(nc) as tc:
        with contextlib.ExitStack() as es:
            const = es.enter_context(tc.tile_pool(name="const", bufs=1))
            meta_p = es.enter_context(tc.tile_pool(name="meta", bufs=1))

            w1_sb = const.tile([D, D], BF16); nc.sync.dma_start(w1_sb[:], W1_in[:])
            w2_sb = const.tile([D, D], BF16); nc.sync.dma_start(w2_sb[:], W2_in[:])
            dinv_sb = const.tile([128, c.NW], F32)
            nc.sync.dma_start(dinv_sb[:], dinv_in[:])
            iota_sb = const.tile([128, 128], BF16)
            nc.sync.dma_start(iota_sb[:], iota_dram[:])
            ident_sb = const.tile([128, 128], BF16)
            nc.sync.dma_start(ident_sb[:], ident_dram[:])
            # gidx first: the first gathers need it; dstloc/xs_own follow on
            # other queues
            gidx_sb = meta_p.tile([128, TOT // 16], I16)
            nc.scalar.dma_start(gidx_sb[:], gidx_in[:])
            dstloc_sb = meta_p.tile([128, TOT_TILES], BF16)
            nc.scalar.dma_start(dstloc_sb[:], dstloc_in[:])
            xs_own_sb = meta_p.tile([128, c.SHARD], BF16)
            nc.sync.dma_start(xs_own_sb[:], xs_own_in[:])

            span_base = {}
            tcol0 = 0
            for b in range(c.NBATCH):
                m0 = 0
                for g in range(c.NGB):
                    span_base[(b, g)] = (tcol0, m0)
                    tcol0 += int(span_tiles[b, g])
                    m0 += int(span_tiles[b, g])

            pre = {}        # (lid, b) -> (dict g -> Mt, set of emitted gbands)
            cc_waited = {1: set(), 2: set(), 3: set()}
            SPANG = [int(span_tiles[:, g].max()) for g in range(c.NGB)]

            def batch_gathers(Mp, lid, b, only_avail=False):
                """Emit (remaining) gathers for batch b of layer lid. Per-band
                M pools: band A (early-available) pipelines deeply without
                waiting for band B's collectives."""
                Mts, done = pre.get((lid, b), ({}, set()))
                for g in range(c.NGB):
                    if g in done or int(span_tiles[b, g]) == 0:
                        continue
                    if lid > 1 and only_avail and any((lid, K) not in ccst
                                                      for K in GB_CHUNKS[g]):
                        continue
                    ntiles = int(span_tiles[b, g])
                    tb, _ = span_base[(b, g)]
                    deps = []
                    if lid > 1 and g not in cc_waited[lid]:
                        deps = [ccst[(lid, K)] for K in GB_CHUNKS[g]]
                        cc_waited[lid].add(g)
                    Mt = Mp.tile([128, SPANG[g], 128], BF16, tag=f"M{g}",
                                 bufs=(8 if g == 0 else 3))
                    # two half-gathers per span: spreads each batch across
                    # all 4 SWDGE queues (desc-gen parallelizes per queue)
                    h1 = (ntiles + 1) // 2
                    for lo, hi in ((0, h1), (h1, ntiles)):
                        if hi > lo:
                            emit_gather(
                                Mt[:, lo:hi, :], tslice(lid, g),
                                gidx_sb[:, (tb + lo) * 8:(tb + hi) * 8],
                                (hi - lo) * 128, deps=deps)
                    Mts[g] = Mt
                    done.add(g)
                pre[(lid, b)] = (Mts, done)
                return Mts

            def build_spans(Sp, b):
                """One wide DVE is_equal per (batch, band) span: all one-hot
                S tiles of the span in a single instruction."""
                Sw = {}
                for g in range(c.NGB):
                    K = int(span_tiles[b, g])
                    if K == 0:
                        continue
                    tb, mb = span_base[(b, g)]
                    St = Sp.tile([128, SPANG[g], 128], BF16, tag=f"S{g}",
                                 bufs=3)
                    nc.vector.tensor_tensor(
                        St[:, :K, :],
                        iota_sb[:].unsqueeze(1).broadcast_to([128, K, 128]),
                        dstloc_sb[:, tb:tb + K].unsqueeze(2)
                            .broadcast_to([128, K, 128]),
                        op=mybir.AluOpType.is_equal)
                    Sw[g] = (St, mb)
                return Sw

            def layer(Mp, lid, hall, hall_next, next_lid):
                """lid==1: aggregate raw xs, then apply W1 (transpose+matmul)
                and W2 per window to produce hs2. lid==2: aggregate hs2,
                relu+scale to produce z."""
                waited_done = []
                with tc.tile_pool(name=f"S{lid}", bufs=6) as Sp, \
                     tc.tile_pool(name=f"ag{lid}", bufs=4, space="PSUM") as agp, \
                     tc.tile_pool(name=f"tp{lid}", bufs=1, space="PSUM") as tpp, \
                     tc.tile_pool(name=f"ep{lid}", bufs=4) as epp:
                    for b in range(c.NBATCH):
                        wlo, whi = b * c.WB, min((b + 1) * c.WB, c.NW)
                        Mts = batch_gathers(Mp, lid, b)
                        Sw = build_spans(Sp, b)
                        for w in range(wlo, whi):
                            ps = agp.tile([128, D], F32, tag="agg")
                            nmm = int(T[w].sum())
                            hsl = hall[:, w * 128:w * 128 + D]
                            nc.tensor.matmul(ps[:], lhsT=ident_sb[:],
                                             rhs=hsl,
                                             start=True, stop=(nmm == 0))
                            mi = 0
                            for g in range(c.NGB):
                                if int(T[w, g]) == 0:
                                    continue
                                St, _ = Sw[g]
                                Mt = Mts[g]
                                off = int(T[wlo:w, g].sum())
                                for t in range(int(T[w, g])):
                                    k = off + t
                                    mi += 1
                                    nc.tensor.matmul(
                                        ps[:], lhsT=St[:, k, :],
                                        rhs=Mt[:, k, :],
                                        start=False,
                                        stop=(mi == nmm))
                            zo = None
                            if lid == 1:
                                # a1 = dinv*(agg+self)  [pre-W1 aggregate]
                                a1 = epp.tile([128, D], BF16, tag="a1")
                                nc.scalar.activation(
                                    a1[:], ps[:], ACTF.Copy,
                                    scale=dinv_sb[:, w:w + 1])
                                t1ps = tpp.tile([128, D], BF16, tag="t1")
                                nc.tensor.transpose(t1ps[:], a1[:], ident_sb[:])
                                a1T = epp.tile([128, D], BF16, tag="a1T")
                                nc.scalar.activation(a1T[:], t1ps[:], ACTF.Copy)
                                yps = tpp.tile([128, D], F32, tag="y")
                                nc.tensor.matmul(yps[:], lhsT=a1T[:],
                                                 rhs=w1_sb[:],
                                                 start=True, stop=True)
                                z = epp.tile([128, D], BF16, tag="z")
                                nc.scalar.activation(z[:], yps[:], ACTF.Relu)
                                t2ps = tpp.tile([128, D], BF16, tag="t2")
                                nc.tensor.transpose(t2ps[:], z[:], ident_sb[:])
                                zT = epp.tile([128, D], BF16, tag="zT")
                                nc.scalar.activation(zT[:], t2ps[:], ACTF.Copy)
                                h2ps = tpp.tile([128, D], F32, tag="h2")
                                nc.tensor.matmul(h2ps[:], lhsT=zT[:],
                                                 rhs=w2_sb[:],
                                                 start=True, stop=True)
                                nc.scalar.activation(
                                    hall_next[:, w * 128:w * 128 + D], h2ps[:],
                                    ACTF.Copy, scale=dinv_sb[:, w:w + 1])
                            else:
                                # layer-2 output only stages toward the shard
                                # DMA; use a small rolling buffer
                                zo = epp.tile([128, D], BF16, tag="zo")
                                nc.scalar.activation(
                                    zo[:], ps[:],
                                    ACTF.Relu, scale=dinv_sb[:, w:w + 1])
                            out_ap = (hall_next[:, w * 128:w * 128 + D]
                                      if lid == 1 else zo[:])
                            write_window(next_lid, w, out_ap, waited_done)
                        if b == c.NBATCH - 1 - c.PREFETCH and next_lid == 2:
                            for pb in range(min(c.PREFETCH + 1, c.NBATCH)):
                                batch_gathers(Mp, 2, pb, only_avail=True)

            with tc.tile_pool(name="hs", bufs=1) as hsp, \
                 tc.tile_pool(name="Mpool", bufs=1) as Mp:
                hs2_all = hsp.tile([128, c.SHARD], BF16, tag="hs2")
                layer(Mp, 1, xs_own_sb, hs2_all, 2)
                layer(Mp, 2, hs2_all, None, 3)

            # decode
            with tc.tile_pool(name="didx", bufs=1) as didxp, \
                 tc.tile_pool(name="dM", bufs=1) as dMp, \
                 tc.tile_pool(name="dw", bufs=6) as dwp, \
                 tc.tile_pool(name="dout", bufs=1) as doutp:
                ds_sb = didxp.tile([128, TOT_DEC // 16], I16)
                nc.scalar.dma_start(ds_sb[:], didx_s_in[:])
                dd_sb = didxp.tile([128, TOT_DEC // 16], I16)
                nc.scalar.dma_start(dd_sb[:], didx_d_in[:])
                Ms = dMp.tile([128, TOT_DEC // 128, D], BF16, tag="Ms")
                Md = dMp.tile([128, TOT_DEC // 128, D], BF16, tag="Md")
                res = doutp.tile([128, TOT_DEC // 128], F32)
                waited = set()
                coff = 0
                for g in gorder:
                    ks, kd = g // c.NGB, g % c.NGB
                    ncols = int(Tdec[g])
                    if ncols == 0:
                        continue
                    dep_s, dep_d = [], []
                    if ks not in waited:
                        dep_s = [ccst[(3, K)] for K in GB_CHUNKS[ks]]
                        waited.add(ks)
                    if kd not in waited:
                        dep_d = [ccst[(3, K)] for K in GB_CHUNKS[kd]]
                        waited.add(kd)
                    h1 = (ncols + 1) // 2
                    for lo, hi in ((0, h1), (h1, ncols)):
                        if hi <= lo:
                            continue
                        emit_gather(Ms[:, coff + lo:coff + hi, :],
                                    tslice(3, ks),
                                    ds_sb[:, (coff + lo) * 8:(coff + hi) * 8],
                                    (hi - lo) * 128, deps=dep_s)
                        emit_gather(Md[:, coff + lo:coff + hi, :],
                                    tslice(3, kd),
                                    dd_sb[:, (coff + lo) * 8:(coff + hi) * 8],
                                    (hi - lo) * 128, deps=dep_d)
                        dep_s, dep_d = [], []
                    # wide elementwise product + innermost-axis reduce: two
                    # DVE ops per group instead of per-tile mult+accum
                    mm = dwp.tile([128, DEC_SPANMAX, 128], F32, tag="mm",
                                  bufs=2)
                    nc.vector.tensor_tensor(
                        mm[:, :ncols, :], Ms[:, coff:coff + ncols, :],
                        Md[:, coff:coff + ncols, :],
                        op=mybir.AluOpType.mult)
                    nc.vector.tensor_reduce(
                        res[:, coff:coff + ncols], mm[:, :ncols, :],
                        axis=mybir.AxisListType.X, op=mybir.AluOpType.add)
                    coff += ncols
                nc.sync.dma_start(dots_out[:], res[:])

    nc.compile()
    return nc


def assemble_output(cfg, meta, results):
    c = cfg
    slot2j = meta["slot2j"]
    out = np.zeros(c.EL, dtype=np.float32)
    for core in range(len(results)):
        d = np.asarray(results[core]["dots"], dtype=np.float32)
        flat = d.T.reshape(-1)             # slot i -> d[i%128, i//128]
        s2j = slot2j[core]
        valid = s2j >= 0
        out[s2j[valid]] = flat[valid]
    return out


def run_pipeline(x, edge_index, edge_label_index, W1, b1, W2, b2,
                 cfg=None, trace=False, tmpdir=None):
    cfg = cfg or DEFAULT
    in_maps, meta = host_prep(cfg, x, edge_index, edge_label_index,
                              W1, b1, W2, b2)
    nc = build_program(cfg, meta)
    res = run_bass_kernel_spmd(nc, in_maps, list(range(cfg.NC)),
                               trace=trace, tmpdir=tmpdir)
    return assemble_output(cfg, meta, res.results), res


def kernel(x, edge_index, edge_label_index, W1, b1, W2, b2):
    out, _ = run_pipeline(x, edge_index, edge_label_index, W1, b1, W2, b2)
    return out


# revision 53
# speedup vs baseline: 1.0852x; 1.0024x over previous
"""Trainium2 Bass kernel for nn_LinkPredictor (2-layer GCN + edge-dot decode).

Strategy (8 NeuronCores, SPMD), v4:
  - Nodes sharded: core c owns rows [c*12544, (c+1)*12544) of the padded
    node table (N=100000 padded to 100352 = 8*98*128).
  - dinv folded into node features: table rows hold hs = dinv[n] * (prev @ W);
    output z = relu(dinv[v]*(agg + hs[v]) + b).
  - Node tables in DRAM are band-major contiguous: collective chunk K holds
    rows c*CH+j of each core's shard; 4 pipelined AllGathers per layer write
    slices of one tensor. Chunk sizes [14,32,32,20] windows: small first
    chunk so layer-1 gathers start early, small last chunk for a short
    decode tail.
  - Gathers use int16 indices relative to per-band bases; band == chunk.
    One dma_gather per (window-batch, gather-band), WB=4 windows per batch.
  - Aggregation: one-hot S built in WIDE batched DVE ops (one tensor_tensor
    is_equal per (batch, band) span using stride-0 broadcast APs) feeding
    PE matmul accumulation into PSUM. Self-loop = identity matmul; relu+dinv
    scale on ScalarE.
  - hs tiles resident in SBUF as wide [128, 12544] tiles; layer-2 output z
    aliases the layer-1 hs tile.
  - Next layer's first batches are prefetched during the current layer's
    tail through a shared M pool.
  - Decode: gathers z[s], z[d] by gather-band pair, one DVE
    tensor_tensor_reduce (mult+add) per tile.
"""
import contextlib
import math
import numpy as np
import ml_dtypes

import concourse.bass as bass
import concourse.tile as tile
from concourse import bacc, mybir
from concourse.bass_utils import run_bass_kernel_spmd
from concourse.tile_rust import add_dep_helper

F32 = mybir.dt.float32
BF16 = mybir.dt.bfloat16
I16 = mybir.dt.int16
BF = ml_dtypes.bfloat16
ACTF = mybir.ActivationFunctionType


class Cfg:
    def __init__(self, N=100000, E=1600000, EL=100000, D=128, ncores=8,
                 nw=98, wb=3, prefetch=3, ch_win=(8, 31, 24, 21, 14),
                 band_split=2):
        self.N, self.E, self.EL, self.D, self.NC = N, E, EL, D, ncores
        self.NW = nw                      # windows (128 nodes each) per core
        self.SHARD = nw * 128             # nodes per core (padded)
        self.NP = self.SHARD * ncores     # padded node count
        assert self.NP >= N
        # collective chunks (rows per core), window-aligned; first chunk
        # small so the first AllGather (and layer-1 gathers) start early,
        # last chunk small so the decode tail is short.
        self.CH_WIN = list(ch_win)        # windows per chunk
        assert sum(self.CH_WIN) == nw
        self.NB = len(self.CH_WIN)
        self.CH_SIZES = [wn * 128 for wn in self.CH_WIN]
        self.CH_STARTS = np.cumsum([0] + self.CH_SIZES).tolist()
        self.TB_SIZES = [s * ncores for s in self.CH_SIZES]
        self.TB_STARTS = np.cumsum([0] + self.TB_SIZES).tolist()
        # 2 gather bands of <=65536 table rows each (signed int16 indices
        # relative to a mid-band base; hardware sign-extends). Band g covers
        # collective chunks GB_CHUNKS[g]; many small chunks keep each
        # AllGather short so layer boundaries don't stall on a big one.
        self.NGB = 2
        bs = self.BAND_SPLIT = band_split
        self.GB_CHUNKS = [list(range(bs)), list(range(bs, self.NB))]
        self.BAND_LO = [self.TB_STARTS[0], self.TB_STARTS[bs]]
        self.BAND_HI = [self.TB_STARTS[bs], self.TB_STARTS[self.NB]]
        for g in range(self.NGB):
            assert self.BAND_HI[g] - self.BAND_LO[g] <= 65536
        self.GBASE = [self.BAND_LO[g] + 32768 for g in range(self.NGB)]
        self.WB = wb                      # windows per gather/aggregate batch
        self.NBATCH = math.ceil(nw / wb)
        self.PREFETCH = prefetch          # next-layer batches gathered early


DEFAULT = Cfg()


def _wrap_idxs(idx):
    """[n] ints -> [128, n//16] int16 wrapped in 16 partitions, replicated 8x."""
    n = len(idx)
    assert n % 16 == 0
    w = np.asarray(idx, dtype=np.int16).reshape(n // 16, 16).T
    return np.ascontiguousarray(np.tile(w, (8, 1)))


def host_prep(cfg, x, edge_index, edge_label_index, W1, b1, W2, b2):
    """All host-side sharding/layout. Returns (per-core input maps, meta)."""
    c = cfg
    src = np.asarray(edge_index[0], dtype=np.int64)
    dst = np.asarray(edge_index[1], dtype=np.int64)
    deg = np.bincount(dst, minlength=c.N).astype(np.float64) + 1.0
    dinv = 1.0 / np.sqrt(deg)                      # [N]
    dinv_p = np.ones(c.NP, dtype=np.float64)
    dinv_p[:c.N] = dinv
    dinv_f = dinv_p.astype(np.float32)

    ch_starts = np.asarray(c.CH_STARTS[:-1])
    def chunk_of(r):
        return np.searchsorted(ch_starts, r, side="right") - 1

    def bmaj_of(n):
        """band-major table row of node id n."""
        cc, r = n // c.SHARD, n % c.SHARD
        K = chunk_of(r)
        return (np.asarray(c.TB_STARTS)[K] + cc * np.asarray(c.CH_SIZES)[K]
                + (r - ch_starts[K]))

    bmaj_src = bmaj_of(src)
    gband_src = (chunk_of(src % c.SHARD) >= c.BAND_SPLIT).astype(np.int64)
    core_of = dst // c.SHARD
    w_of = (dst % c.SHARD) // 128
    dloc = dst % 128

    key = (core_of * c.NW + w_of) * c.NGB + gband_src
    ngroups = c.NC * c.NW * c.NGB
    order = np.argsort(key, kind="stable")
    counts = np.bincount(key, minlength=ngroups).reshape(c.NC, c.NW, c.NGB)
    starts = np.zeros(ngroups + 1, dtype=np.int64)
    np.cumsum(np.bincount(key, minlength=ngroups), out=starts[1:])

    T = np.ceil(counts.max(axis=0) / 128).astype(np.int64)     # [NW, NGB]
    TOT_TILES = int(T.sum())
    span_tiles = np.zeros((c.NBATCH, c.NGB), dtype=np.int64)
    for b in range(c.NBATCH):
        wlo, whi = b * c.WB, min((b + 1) * c.WB, c.NW)
        for g in range(c.NGB):
            span_tiles[b, g] = T[wlo:whi, g].sum()
    TOT = TOT_TILES * 128

    idx_arr = np.zeros((c.NC, TOT), dtype=np.int64)
    dloc_arr = np.full((c.NC, TOT), -1.0, dtype=np.float32)
    for core in range(c.NC):
        pos = 0
        for b in range(c.NBATCH):
            wlo, whi = b * c.WB, min((b + 1) * c.WB, c.NW)
            for g in range(c.NGB):
                grp_start = pos
                for w in range(wlo, whi):
                    gk = (core * c.NW + w) * c.NGB + g
                    eids = order[starts[gk]:starts[gk + 1]]
                    n = len(eids)
                    idx_arr[core, pos:pos + n] = bmaj_src[eids] - c.GBASE[g]
                    dloc_arr[core, pos:pos + n] = dloc[eids]
                    grp_start = pos
                    pos += int(T[w, g]) * 128
                # Each (b, g) span is gathered as TWO half-gathers (to spread
                # across SWDGE queues). The ucode strips TRAILING negative
                # indices from each gather: the final slot of each half must
                # be >= 0. Swap within the window group containing that slot
                # (slots in a group share the same dst window, so any
                # permutation is safe).
                span_lo = pos - int(span_tiles[b, g]) * 128
                K = int(span_tiles[b, g])
                h1 = (K + 1) // 2
                ends = ([pos - 1] if K > 0 else []) + \
                       ([span_lo + h1 * 128 - 1] if 0 < h1 < K else [])
                for endslot in ends:
                    if idx_arr[core, endslot] >= 0:
                        continue
                    # find the (w,g) group containing endslot
                    gl = span_lo
                    for w in range(wlo, whi):
                        gh = gl + int(T[w, g]) * 128
                        if gl <= endslot < gh:
                            break
                        gl = gh
                    cand = np.nonzero(idx_arr[core, gl:gh] >= 0)[0] + gl
                    cand = [j for j in cand if j not in ends]
                    assert len(cand) > 0, "all-negative group"
                    j = cand[0]
                    for arr in (idx_arr, dloc_arr):
                        arr[core, j], arr[core, endslot] = \
                            arr[core, endslot], arr[core, j]
        assert pos == TOT
    assert idx_arr.min() >= -32768 and idx_arr.max() < 32768

    # decode: label edge j -> core j // ELC; groups by (gband(s), gband(d))
    assert c.EL % c.NC == 0
    ELC = c.EL // c.NC
    ls = np.asarray(edge_label_index[0], dtype=np.int64)
    ld = np.asarray(edge_label_index[1], dtype=np.int64)
    bs, bd = bmaj_of(ls), bmaj_of(ld)
    gs = (chunk_of(ls % c.SHARD) >= c.BAND_SPLIT).astype(np.int64)
    gd = (chunk_of(ld % c.SHARD) >= c.BAND_SPLIT).astype(np.int64)
    gdec = gs * c.NGB + gd
    NG_DEC = c.NGB * c.NGB
    cnt_dec = np.zeros((c.NC, NG_DEC), dtype=np.int64)
    for core in range(c.NC):
        cnt_dec[core] = np.bincount(gdec[core * ELC:(core + 1) * ELC],
                                    minlength=NG_DEC)
    Tdec = np.ceil(cnt_dec.max(axis=0) / 128).astype(np.int64)   # [NG_DEC]
    gorder = sorted(range(NG_DEC), key=lambda g: (max(g // c.NGB, g % c.NGB), g))
    TOT_DEC = int(Tdec.sum()) * 128
    idx_s = np.zeros((c.NC, TOT_DEC), dtype=np.int64)
    idx_d = np.zeros((c.NC, TOT_DEC), dtype=np.int64)
    slot2j = np.full((c.NC, TOT_DEC), -1, dtype=np.int64)
    for core in range(c.NC):
        jlo = core * ELC
        kk = gdec[jlo:jlo + ELC]
        o = np.argsort(kk, kind="stable")
        st = np.zeros(NG_DEC + 1, dtype=np.int64)
        np.cumsum(np.bincount(kk, minlength=NG_DEC), out=st[1:])
        pos = 0
        for g in gorder:
            js = o[st[g]:st[g + 1]] + jlo
            n = len(js)
            idx_s[core, pos:pos + n] = bs[js] - c.GBASE[g // c.NGB]
            idx_d[core, pos:pos + n] = bd[js] - c.GBASE[g % c.NGB]
            slot2j[core, pos:pos + n] = js
            p1 = pos + int(Tdec[g]) * 128
            # keep the final slot of each decode half-gather non-negative
            # (the ucode strips trailing negative indices); decode slots are
            # freely permutable within a group (slot2j carries the mapping)
            Kd = int(Tdec[g])
            h1 = (Kd + 1) // 2
            ends = ([p1 - 1] if Kd else []) + \
                   ([pos + h1 * 128 - 1] if 0 < h1 < Kd else [])
            for endslot in ends:
                if idx_s[core, endslot] >= 0 and idx_d[core, endslot] >= 0:
                    continue
                ok = np.nonzero((idx_s[core, pos:p1] >= 0) &
                                (idx_d[core, pos:p1] >= 0))[0] + pos
                ok = [j for j in ok if j not in ends]
                assert len(ok) > 0
                j = ok[0]
                for a2 in (idx_s, idx_d, slot2j):
                    a2[core, j], a2[core, endslot] = \
                        a2[core, endslot], a2[core, j]
            pos = p1
        assert pos == TOT_DEC

    xp = np.zeros((c.NP, c.D), dtype=np.float32)
    xp[:c.N] = np.asarray(x, dtype=np.float32)
    use_b1 = bool(np.any(np.asarray(b1)))
    use_b2 = bool(np.any(np.asarray(b2)))
    assert not (use_b1 or use_b2), "bias path not wired in v10"

    # xs = dinv * x, in band-major table order: layer-1 gathers read this
    # host-provided table directly -> no P0 / no table-1 collectives.
    xs = xp * dinv_f[:, None]
    perm = bmaj_of(np.arange(c.NP))        # node id -> table row
    xs_tab = np.zeros((c.NP, c.D), dtype=np.float32)
    xs_tab[perm] = xs
    xs_tab = np.ascontiguousarray(xs_tab).astype(BF)

    in_maps = []
    for core in range(c.NC):
        sl = slice(core * c.SHARD, (core + 1) * c.SHARD)
        dsh = dinv_f[sl]
        m = {
            "xs_tab": xs_tab,
            "xs_own": np.ascontiguousarray(
                xs[sl].reshape(c.NW, 128, c.D).transpose(1, 0, 2)
                .reshape(128, c.NW * c.D)).astype(BF),
            "W1": np.asarray(W1, dtype=np.float32).astype(BF),
            "W2": np.asarray(W2, dtype=np.float32).astype(BF),
            "dinv": np.ascontiguousarray(dsh.reshape(c.NW, 128).T),
            "gidx": _wrap_idxs(idx_arr[core]),
            "dstloc": np.ascontiguousarray(
                dloc_arr[core].reshape(TOT_TILES, 128).T).astype(BF),
            "didx_s": _wrap_idxs(idx_s[core]),
            "didx_d": _wrap_idxs(idx_d[core]),
        }
        in_maps.append(m)
    meta = dict(T=T, span_tiles=span_tiles, TOT=TOT, TOT_TILES=TOT_TILES,
                Tdec=Tdec, gorder=gorder, TOT_DEC=TOT_DEC, slot2j=slot2j,
                use_b1=use_b1, use_b2=use_b2)
    return in_maps, meta


def build_program(cfg, meta, num_cores=None):
    c = cfg
    NCores = num_cores or c.NC
    T, span_tiles = meta["T"], meta["span_tiles"]
    TOT, TOT_TILES = meta["TOT"], meta["TOT_TILES"]
    Tdec, gorder, TOT_DEC = meta["Tdec"], meta["gorder"], meta["TOT_DEC"]
    use_b = {1: meta["use_b1"], 2: meta["use_b2"]}
    D = c.D
    TBMAX = int(span_tiles.sum(axis=1).max())
    SPANMAX = int(span_tiles.max())
    DEC_SPANMAX = int(Tdec.max())
    GB_CHUNKS = c.GB_CHUNKS

    nc = bacc.Bacc("TRN2", target_bir_lowering=False, debug=False,
                   num_devices=NCores, num_swdge_queues=4)
    NQ = 4

    assert not (use_b[1] or use_b[2])
    xs_tab_in = nc.dram_tensor("xs_tab", [c.TB_STARTS[-1], D], BF16,
                               kind="ExternalInput")
    xs_own_in = nc.dram_tensor("xs_own", [128, c.SHARD], BF16,
                               kind="ExternalInput")
    W1_in = nc.dram_tensor("W1", [D, D], BF16, kind="ExternalInput")
    W2_in = nc.dram_tensor("W2", [D, D], BF16, kind="ExternalInput")
    dinv_in = nc.dram_tensor("dinv", [128, c.NW], F32, kind="ExternalInput")
    gidx_in = nc.dram_tensor("gidx", [128, TOT // 16], I16, kind="ExternalInput")
    dstloc_in = nc.dram_tensor("dstloc", [128, TOT_TILES], BF16, kind="ExternalInput")
    didx_s_in = nc.dram_tensor("didx_s", [128, TOT_DEC // 16], I16, kind="ExternalInput")
    didx_d_in = nc.dram_tensor("didx_d", [128, TOT_DEC // 16], I16, kind="ExternalInput")
    dots_out = nc.dram_tensor("dots", [128, TOT_DEC // 128], F32, kind="ExternalOutput")

    shard_b = {l: [nc.dram_tensor(f"shard{l}_{k}", [c.CH_SIZES[k], D], BF16)
                   for k in range(c.NB)] for l in (2, 3)}
    table = {l: nc.dram_tensor(f"table{l}", [c.TB_STARTS[-1], D], BF16,
                               addr_space="Shared") for l in (2, 3)}

    def tslice(l, g):
        """Gather source AP for band g: base at GBASE[g] (mid-band); signed
        int16 indices reach the whole band [BAND_LO, BAND_HI). Layer 1
        gathers the host-provided xs table (no collective needed)."""
        src = xs_tab_in if l == 1 else table[l]
        return src[c.GBASE[g]:c.BAND_HI[g], :]

    iota_dram = nc.inline_tensor(
        np.tile(np.arange(128, dtype=np.float32), (128, 1)).astype(BF), "iota_c")
    ident_dram = nc.inline_tensor(np.eye(128, dtype=np.float32).astype(BF), "ident_c")

    core_ids = list(range(NCores))
    gst = {"count": 0, "prev": None}
    ccst = {}                            # (l, K) -> collective instruction

    def emit_gather(out_ap, in_ap, idx_ap, n_idx, deps=()):
        q = gst["count"] % NQ
        inst = nc.gpsimd.dma_gather(out_ap, in_ap, idx_ap, n_idx, n_idx, D,
                                    queue_num=q, single_packet=False)
        for dcc in deps:
            add_dep_helper(inst.ins, dcc.ins, sync=True,
                           reason="gather after collective")
        gst["count"] += 1
        return inst

    def emit_collective(l, K):
        cc = nc.gpsimd.collective_compute(
            "AllGather", mybir.AluOpType.bypass,
            replica_groups=[core_ids],
            ins=[shard_b[l][K][:]],
            outs=[table[l][c.TB_STARTS[K]:c.TB_STARTS[K + 1], :]],
        )
        ccst[(l, K)] = cc
        return cc

    def write_window(l, w, src_ap, done_k):
        """DMA window w rows into its chunk shard; fire collectives when a
        chunk completes (chunks are window-aligned)."""
        lo = w * 128
        K = 0
        while c.CH_STARTS[K + 1] <= lo:
            K += 1
        off = lo - c.CH_STARTS[K]
        nc.sync.dma_start(shard_b[l][K][off:off + 128, :], src_ap)
        while len(done_k) < c.NB and \
                (w + 1) * 128 >= c.CH_STARTS[len(done_k) + 1]:
            emit_collective(l, len(done_k))
            done_k.append(len(done_k))

    with tile.TileContext(nc) as tc:
        with contextlib.ExitStack() as es:
            const = es.enter_context(tc.tile_pool(name="const", bufs=1))
            meta_p = es.enter_context(tc.tile_pool(name="meta", bufs=1))

            w1_sb = const.tile([D, D], BF16); nc.sync.dma_start(w1_sb[:], W1_in[:])
            w2_sb = const.tile([D, D], BF16); nc.sync.dma_start(w2_sb[:], W2_in[:])
            dinv_sb = const.tile([128, c.NW], F32)
            nc.sync.dma_start(dinv_sb[:], dinv_in[:])
            iota_sb = const.tile([128, 128], BF16)
            nc.sync.dma_start(iota_sb[:], iota_dram[:])
            ident_sb = const.tile([128, 128], BF16)
            nc.sync.dma_start(ident_sb[:], ident_dram[:])
            # gidx first: the first gathers need it; dstloc/xs_own follow on
            # other queues
            gidx_sb = meta_p.tile([128, TOT // 16], I16)
            nc.scalar.dma_start(gidx_sb[:], gidx_in[:])
            dstloc_sb = meta_p.tile([128, TOT_TILES], BF16)
            nc.scalar.dma_start(dstloc_sb[:], dstloc_in[:])
            xs_own_sb = meta_p.tile([128, c.SHARD], BF16)
            nc.sync.dma_start(xs_own_sb[:], xs_own_in[:])

            span_base = {}
            tcol0 = 0
            for b in range(c.NBATCH):
                m0 = 0
                for g in range(c.NGB):
                    span_base[(b, g)] = (tcol0, m0)
                    tcol0 += int(span_tiles[b, g])
                    m0 += int(span_tiles[b, g])

            pre = {}        # (lid, b) -> (dict g -> Mt, set of emitted gbands)
            cc_waited = {1: set(), 2: set(), 3: set()}
            SPANG = [int(span_tiles[:, g].max()) for g in range(c.NGB)]

            def batch_gathers(Mp, lid, b, only_avail=False):
                """Emit (remaining) gathers for batch b of layer lid. Per-band
                M pools: band A (early-available) pipelines deeply without
                waiting for band B's collectives."""
                Mts, done = pre.get((lid, b), ({}, set()))
                for g in range(c.NGB):
                    if g in done or int(span_tiles[b, g]) == 0:
                        continue
                    if lid > 1 and only_avail and any((lid, K) not in ccst
                                                      for K in GB_CHUNKS[g]):
                        continue
                    ntiles = int(span_tiles[b, g])
                    tb, _ = span_base[(b, g)]
                    deps = []
                    if lid > 1 and g not in cc_waited[lid]:
                        deps = [ccst[(lid, K)] for K in GB_CHUNKS[g]]
                        cc_waited[lid].add(g)
                    Mt = Mp.tile([128, SPANG[g], 128], BF16, tag=f"M{g}",
                                 bufs=(8 if g == 0 else 3))
                    # two half-gathers per span: spreads each batch across
                    # all 4 SWDGE queues (desc-gen parallelizes per queue)
                    h1 = (ntiles + 1) // 2
                    for lo, hi in ((0, h1), (h1, ntiles)):
                        if hi > lo:
                            emit_gather(
                                Mt[:, lo:hi, :], tslice(lid, g),
                                gidx_sb[:, (tb + lo) * 8:(tb + hi) * 8],
                                (hi - lo) * 128, deps=deps)
                    Mts[g] = Mt
                    done.add(g)
                pre[(lid, b)] = (Mts, done)
                return Mts

            def build_spans(Sp, b):
                """One wide DVE is_equal per (batch, band) span: all one-hot
                S tiles of the span in a single instruction."""
                Sw = {}
                for g in range(c.NGB):
                    K = int(span_tiles[b, g])
                    if K == 0:
                        continue
                    tb, mb = span_base[(b, g)]
                    St = Sp.tile([128, SPANG[g], 128], BF16, tag=f"S{g}",
                                 bufs=3)
                    nc.vector.tensor_tensor(
                        St[:, :K, :],
                        iota_sb[:].unsqueeze(1).broadcast_to([128, K, 128]),
                        dstloc_sb[:, tb:tb + K].unsqueeze(2)
                            .broadcast_to([128, K, 128]),
                        op=mybir.AluOpType.is_equal)
                    Sw[g] = (St, mb)
                return Sw

            def layer(Mp, lid, hall, hall_next, next_lid):
                """lid==1: aggregate raw xs, then apply W1 (transpose+matmul)
                and W2 per window to produce hs2. lid==2: aggregate hs2,
                relu+scale to produce z."""
                waited_done = []
                with tc.tile_pool(name=f"S{lid}", bufs=6) as Sp, \
                     tc.tile_pool(name=f"ag{lid}", bufs=4, space="PSUM") as agp, \
                     tc.tile_pool(name=f"tp{lid}", bufs=1, space="PSUM") as tpp, \
                     tc.tile_pool(name=f"ep{lid}", bufs=4) as epp:
                    for b in range(c.NBATCH):
                        wlo, whi = b * c.WB, min((b + 1) * c.WB, c.NW)
                        Mts = batch_gathers(Mp, lid, b)
                        Sw = build_spans(Sp, b)
                        for w in range(wlo, whi):
                            ps = agp.tile([128, D], F32, tag="agg")
                            nmm = int(T[w].sum())
                            hsl = hall[:, w * 128:w * 128 + D]
                            nc.tensor.matmul(ps[:], lhsT=ident_sb[:],
                                             rhs=hsl,
                                             start=True, stop=(nmm == 0))
                            mi = 0
                            for g in range(c.NGB):
                                if int(T[w, g]) == 0:
                                    continue
                                St, _ = Sw[g]
                                Mt = Mts[g]
                                off = int(T[wlo:w, g].sum())
                                for t in range(int(T[w, g])):
                                    k = off + t
                                    mi += 1
                                    nc.tensor.matmul(
                                        ps[:], lhsT=St[:, k, :],
                                        rhs=Mt[:, k, :],
                                        start=False,
                                        stop=(mi == nmm))
                            zo = None
                            if lid == 1:
                                # a1 = dinv*(agg+self)  [pre-W1 aggregate]
                                a1 = epp.tile([128, D], BF16, tag="a1")
                                nc.scalar.activation(
                                    a1[:], ps[:], ACTF.Copy,
                                    scale=dinv_sb[:, w:w + 1])
                                t1ps = tpp.tile([128, D], BF16, tag="t1")
                                nc.tensor.transpose(t1ps[:], a1[:], ident_sb[:])
                                a1T = epp.tile([128, D], BF16, tag="a1T")
                                nc.scalar.activation(a1T[:], t1ps[:], ACTF.Copy)
                                yps = tpp.tile([128, D], F32, tag="y")
                                nc.tensor.matmul(yps[:], lhsT=a1T[:],
                                                 rhs=w1_sb[:],
                                                 start=True, stop=True)
                                z = epp.tile([128, D], BF16, tag="z")
                                nc.scalar.activation(z[:], yps[:], ACTF.Relu)
                                t2ps = tpp.tile([128, D], BF16, tag="t2")
                                nc.tensor.transpose(t2ps[:], z[:], ident_sb[:])
                                zT = epp.tile([128, D], BF16, tag="zT")
                                nc.scalar.activation(zT[:], t2ps[:], ACTF.Copy)
                                h2ps = tpp.tile([128, D], F32, tag="h2")
                                nc.tensor.matmul(h2ps[:], lhsT=zT[:],
                                                 rhs=w2_sb[:],
                                                 start=True, stop=True)
                                nc.scalar.activation(
                                    hall_next[:, w * 128:w * 128 + D], h2ps[:],
                                    ACTF.Copy, scale=dinv_sb[:, w:w + 1])
                            else:
                                # layer-2 output only stages toward the shard
                                # DMA; use a small rolling buffer
                                zo = epp.tile([128, D], BF16, tag="zo")
                                nc.scalar.activation(
                                    zo[:], ps[:],
                                    ACTF.Relu, scale=dinv_sb[:, w:w + 1])
                            out_ap = (hall_next[:, w * 128:w * 128 + D]
                                      if lid == 1 else zo[:])
                            write_window(next_lid, w, out_ap, waited_done)
                        if b == c.NBATCH - 1 - c.PREFETCH and next_lid == 2:
                            for pb in range(min(c.PREFETCH + 1, c.NBATCH)):
                                batch_gathers(Mp, 2, pb, only_avail=True)

            with tc.tile_pool(name="hs", bufs=1) as hsp, \
                 tc.tile_pool(name="Mpool", bufs=1) as Mp:
                hs2_all = hsp.tile([128, c.SHARD], BF16, tag="hs2")
                layer(Mp, 1, xs_own_sb, hs2_all, 2)
                layer(Mp, 2, hs2_all, None, 3)

            # decode
            with tc.tile_pool(name="didx", bufs=1) as didxp, \
                 tc.tile_pool(name="dM", bufs=1) as dMp, \
                 tc.tile_pool(name="dw", bufs=6) as dwp, \
                 tc.tile_pool(name="dout", bufs=1) as doutp:
                ds_sb = didxp.tile([128, TOT_DEC // 16], I16)
                nc.scalar.dma_start(ds_sb[:], didx_s_in[:])
                dd_sb = didxp.tile([128, TOT_DEC // 16], I16)
                nc.scalar.dma_start(dd_sb[:], didx_d_in[:])
                Ms = dMp.tile([128, TOT_DEC // 128, D], BF16, tag="Ms")
                Md = dMp.tile([128, TOT_DEC // 128, D], BF16, tag="Md")
                res = doutp.tile([128, TOT_DEC // 128], F32)
                waited = set()
                coff = 0
                for g in gorder:
                    ks, kd = g // c.NGB, g % c.NGB
                    ncols = int(Tdec[g])
                    if ncols == 0:
                        continue
                    dep_s, dep_d = [], []
                    if ks not in waited:
                        dep_s = [ccst[(3, K)] for K in GB_CHUNKS[ks]]
                        waited.add(ks)
                    if kd not in waited:
                        dep_d = [ccst[(3, K)] for K in GB_CHUNKS[kd]]
                        waited.add(kd)
                    h1 = (ncols + 1) // 2
                    for lo, hi in ((0, h1), (h1, ncols)):
                        if hi <= lo:
                            continue
                        emit_gather(Ms[:, coff + lo:coff + hi, :],
                                    tslice(3, ks),
                                    ds_sb[:, (coff + lo) * 8:(coff + hi) * 8],
                                    (hi - lo) * 128, deps=dep_s)
                        emit_gather(Md[:, coff + lo:coff + hi, :],
                                    tslice(3, kd),
                                    dd_sb[:, (coff + lo) * 8:(coff + hi) * 8],
                                    (hi - lo) * 128, deps=dep_d)
                        dep_s, dep_d = [], []
                    # wide elementwise product + innermost-axis reduce: two
                    # DVE ops per group instead of per-tile mult+accum
                    mm = dwp.tile([128, DEC_SPANMAX, 128], F32, tag="mm",
                                  bufs=2)
                    nc.vector.tensor_tensor(
                        mm[:, :ncols, :], Ms[:, coff:coff + ncols, :],
                        Md[:, coff:coff + ncols, :],
                        op=mybir.AluOpType.mult)
                    nc.vector.tensor_reduce(
                        res[:, coff:coff + ncols], mm[:, :ncols, :],
                        axis=mybir.AxisListType.X, op=mybir.AluOpType.add)
                    coff += ncols
                nc.sync.dma_start(dots_out[:], res[:])

    nc.compile()
    return nc


def assemble_output(cfg, meta, results):
    c = cfg
    slot2j = meta["slot2j"]
    out = np.zeros(c.EL, dtype=np.float32)
    for core in range(len(results)):
        d = np.asarray(results[core]["dots"], dtype=np.float32)
        flat = d.T.reshape(-1)             # slot i -> d[i%128, i//128]
        s2j = slot2j[core]
        valid = s2j >= 0
        out[s2j[valid]] = flat[valid]
    return out


def run_pipeline(x, edge_index, edge_label_index, W1, b1, W2, b2,
                 cfg=None, trace=False, tmpdir=None):
    cfg = cfg or DEFAULT
    in_maps, meta = host_prep(cfg, x, edge_index, edge_label_index,
                              W1, b1, W2, b2)
    nc = build_program(cfg, meta)
    res = run_bass_kernel_spmd(nc, in_maps, list(range(cfg.NC)),
                               trace=trace, tmpdir=tmpdir)
    return assemble_output(cfg, meta, res.results), res


def kernel(x, edge_index, edge_label_index, W1, b1, W2, b2):
    out, _ = run_pipeline(x, edge_index, edge_label_index, W1, b1, W2, b2)
    return out


# revision 54
# speedup vs baseline: 1.1203x; 1.0324x over previous
"""Trainium2 Bass kernel for nn_LinkPredictor (2-layer GCN + edge-dot decode).

Strategy (8 NeuronCores, SPMD), v4:
  - Nodes sharded: core c owns rows [c*12544, (c+1)*12544) of the padded
    node table (N=100000 padded to 100352 = 8*98*128).
  - dinv folded into node features: table rows hold hs = dinv[n] * (prev @ W);
    output z = relu(dinv[v]*(agg + hs[v]) + b).
  - Node tables in DRAM are band-major contiguous: collective chunk K holds
    rows c*CH+j of each core's shard; 4 pipelined AllGathers per layer write
    slices of one tensor. Chunk sizes [14,32,32,20] windows: small first
    chunk so layer-1 gathers start early, small last chunk for a short
    decode tail.
  - Gathers use int16 indices relative to per-band bases; band == chunk.
    One dma_gather per (window-batch, gather-band), WB=4 windows per batch.
  - Aggregation: one-hot S built in WIDE batched DVE ops (one tensor_tensor
    is_equal per (batch, band) span using stride-0 broadcast APs) feeding
    PE matmul accumulation into PSUM. Self-loop = identity matmul; relu+dinv
    scale on ScalarE.
  - hs tiles resident in SBUF as wide [128, 12544] tiles; layer-2 output z
    aliases the layer-1 hs tile.
  - Next layer's first batches are prefetched during the current layer's
    tail through a shared M pool.
  - Decode: gathers z[s], z[d] by gather-band pair, one DVE
    tensor_tensor_reduce (mult+add) per tile.
"""
import contextlib
import math
import numpy as np
import ml_dtypes

import concourse.bass as bass
import concourse.tile as tile
from concourse import bacc, mybir
from concourse.bass_utils import run_bass_kernel_spmd
from concourse.tile_rust import add_dep_helper

F32 = mybir.dt.float32
BF16 = mybir.dt.bfloat16
I16 = mybir.dt.int16
BF = ml_dtypes.bfloat16
ACTF = mybir.ActivationFunctionType


class Cfg:
    def __init__(self, N=100000, E=1600000, EL=100000, D=128, ncores=8,
                 nw=98, wb=3, prefetch=3, ch_win=(8, 31, 24, 21, 14),
                 band_split=2):
        self.N, self.E, self.EL, self.D, self.NC = N, E, EL, D, ncores
        self.NW = nw                      # windows (128 nodes each) per core
        self.SHARD = nw * 128             # nodes per core (padded)
        self.NP = self.SHARD * ncores     # padded node count
        assert self.NP >= N
        # collective chunks (rows per core), window-aligned; first chunk
        # small so the first AllGather (and layer-1 gathers) start early,
        # last chunk small so the decode tail is short.
        self.CH_WIN = list(ch_win)        # windows per chunk
        assert sum(self.CH_WIN) == nw
        self.NB = len(self.CH_WIN)
        self.CH_SIZES = [wn * 128 for wn in self.CH_WIN]
        self.CH_STARTS = np.cumsum([0] + self.CH_SIZES).tolist()
        self.TB_SIZES = [s * ncores for s in self.CH_SIZES]
        self.TB_STARTS = np.cumsum([0] + self.TB_SIZES).tolist()
        # 2 gather bands of <=65536 table rows each (signed int16 indices
        # relative to a mid-band base; hardware sign-extends). Band g covers
        # collective chunks GB_CHUNKS[g]; many small chunks keep each
        # AllGather short so layer boundaries don't stall on a big one.
        self.NGB = 2
        bs = self.BAND_SPLIT = band_split
        self.GB_CHUNKS = [list(range(bs)), list(range(bs, self.NB))]
        self.BAND_LO = [self.TB_STARTS[0], self.TB_STARTS[bs]]
        self.BAND_HI = [self.TB_STARTS[bs], self.TB_STARTS[self.NB]]
        for g in range(self.NGB):
            assert self.BAND_HI[g] - self.BAND_LO[g] <= 65536
        self.GBASE = [self.BAND_LO[g] + 32768 for g in range(self.NGB)]
        self.WB = wb                      # windows per gather/aggregate batch
        self.NBATCH = math.ceil(nw / wb)
        self.PREFETCH = prefetch          # next-layer batches gathered early


DEFAULT = Cfg()


def _wrap_idxs(idx):
    """[n] ints -> [128, n//16] int16 wrapped in 16 partitions, replicated 8x."""
    n = len(idx)
    assert n % 16 == 0
    w = np.asarray(idx, dtype=np.int16).reshape(n // 16, 16).T
    return np.ascontiguousarray(np.tile(w, (8, 1)))


def host_prep(cfg, x, edge_index, edge_label_index, W1, b1, W2, b2):
    """All host-side sharding/layout. Returns (per-core input maps, meta)."""
    c = cfg
    src = np.asarray(edge_index[0], dtype=np.int64)
    dst = np.asarray(edge_index[1], dtype=np.int64)
    deg = np.bincount(dst, minlength=c.N).astype(np.float64) + 1.0
    dinv = 1.0 / np.sqrt(deg)                      # [N]
    dinv_p = np.ones(c.NP, dtype=np.float64)
    dinv_p[:c.N] = dinv
    dinv_f = dinv_p.astype(np.float32)

    ch_starts = np.asarray(c.CH_STARTS[:-1])
    def chunk_of(r):
        return np.searchsorted(ch_starts, r, side="right") - 1

    def bmaj_of(n):
        """band-major table row of node id n."""
        cc, r = n // c.SHARD, n % c.SHARD
        K = chunk_of(r)
        return (np.asarray(c.TB_STARTS)[K] + cc * np.asarray(c.CH_SIZES)[K]
                + (r - ch_starts[K]))

    bmaj_src = bmaj_of(src)
    gband_src = (chunk_of(src % c.SHARD) >= c.BAND_SPLIT).astype(np.int64)
    core_of = dst // c.SHARD
    w_of = (dst % c.SHARD) // 128
    dloc = dst % 128

    key = (core_of * c.NW + w_of) * c.NGB + gband_src
    ngroups = c.NC * c.NW * c.NGB
    order = np.argsort(key, kind="stable")
    counts = np.bincount(key, minlength=ngroups).reshape(c.NC, c.NW, c.NGB)
    starts = np.zeros(ngroups + 1, dtype=np.int64)
    np.cumsum(np.bincount(key, minlength=ngroups), out=starts[1:])

    T = np.ceil(counts.max(axis=0) / 128).astype(np.int64)     # [NW, NGB]
    TOT_TILES = int(T.sum())
    span_tiles = np.zeros((c.NBATCH, c.NGB), dtype=np.int64)
    for b in range(c.NBATCH):
        wlo, whi = b * c.WB, min((b + 1) * c.WB, c.NW)
        for g in range(c.NGB):
            span_tiles[b, g] = T[wlo:whi, g].sum()
    TOT = TOT_TILES * 128

    idx_arr = np.zeros((c.NC, TOT), dtype=np.int64)
    dloc_arr = np.full((c.NC, TOT), -1.0, dtype=np.float32)
    for core in range(c.NC):
        pos = 0
        for b in range(c.NBATCH):
            wlo, whi = b * c.WB, min((b + 1) * c.WB, c.NW)
            for g in range(c.NGB):
                grp_start = pos
                for w in range(wlo, whi):
                    gk = (core * c.NW + w) * c.NGB + g
                    eids = order[starts[gk]:starts[gk + 1]]
                    n = len(eids)
                    idx_arr[core, pos:pos + n] = bmaj_src[eids] - c.GBASE[g]
                    dloc_arr[core, pos:pos + n] = dloc[eids]
                    grp_start = pos
                    pos += int(T[w, g]) * 128
                # Each (b, g) span is gathered as TWO half-gathers (to spread
                # across SWDGE queues). The ucode strips TRAILING negative
                # indices from each gather: the final slot of each half must
                # be >= 0. Swap within the window group containing that slot
                # (slots in a group share the same dst window, so any
                # permutation is safe).
                span_lo = pos - int(span_tiles[b, g]) * 128
                K = int(span_tiles[b, g])
                h1 = (K + 1) // 2
                ends = ([pos - 1] if K > 0 else []) + \
                       ([span_lo + h1 * 128 - 1] if 0 < h1 < K else [])
                for endslot in ends:
                    if idx_arr[core, endslot] >= 0:
                        continue
                    # find the (w,g) group containing endslot
                    gl = span_lo
                    for w in range(wlo, whi):
                        gh = gl + int(T[w, g]) * 128
                        if gl <= endslot < gh:
                            break
                        gl = gh
                    cand = np.nonzero(idx_arr[core, gl:gh] >= 0)[0] + gl
                    cand = [j for j in cand if j not in ends]
                    assert len(cand) > 0, "all-negative group"
                    j = cand[0]
                    for arr in (idx_arr, dloc_arr):
                        arr[core, j], arr[core, endslot] = \
                            arr[core, endslot], arr[core, j]
        assert pos == TOT
    assert idx_arr.min() >= -32768 and idx_arr.max() < 32768

    # decode: label edge j -> core j // ELC; groups by (gband(s), gband(d))
    assert c.EL % c.NC == 0
    ELC = c.EL // c.NC
    ls = np.asarray(edge_label_index[0], dtype=np.int64)
    ld = np.asarray(edge_label_index[1], dtype=np.int64)
    bs, bd = bmaj_of(ls), bmaj_of(ld)
    gs = (chunk_of(ls % c.SHARD) >= c.BAND_SPLIT).astype(np.int64)
    gd = (chunk_of(ld % c.SHARD) >= c.BAND_SPLIT).astype(np.int64)
    gdec = gs * c.NGB + gd
    NG_DEC = c.NGB * c.NGB
    cnt_dec = np.zeros((c.NC, NG_DEC), dtype=np.int64)
    for core in range(c.NC):
        cnt_dec[core] = np.bincount(gdec[core * ELC:(core + 1) * ELC],
                                    minlength=NG_DEC)
    Tdec = np.ceil(cnt_dec.max(axis=0) / 128).astype(np.int64)   # [NG_DEC]
    gorder = sorted(range(NG_DEC), key=lambda g: (max(g // c.NGB, g % c.NGB), g))
    TOT_DEC = int(Tdec.sum()) * 128
    idx_s = np.zeros((c.NC, TOT_DEC), dtype=np.int64)
    idx_d = np.zeros((c.NC, TOT_DEC), dtype=np.int64)
    slot2j = np.full((c.NC, TOT_DEC), -1, dtype=np.int64)
    for core in range(c.NC):
        jlo = core * ELC
        kk = gdec[jlo:jlo + ELC]
        o = np.argsort(kk, kind="stable")
        st = np.zeros(NG_DEC + 1, dtype=np.int64)
        np.cumsum(np.bincount(kk, minlength=NG_DEC), out=st[1:])
        pos = 0
        for g in gorder:
            js = o[st[g]:st[g + 1]] + jlo
            n = len(js)
            idx_s[core, pos:pos + n] = bs[js] - c.GBASE[g // c.NGB]
            idx_d[core, pos:pos + n] = bd[js] - c.GBASE[g % c.NGB]
            slot2j[core, pos:pos + n] = js
            p1 = pos + int(Tdec[g]) * 128
            # keep the final slot of each decode half-gather non-negative
            # (the ucode strips trailing negative indices); decode slots are
            # freely permutable within a group (slot2j carries the mapping)
            Kd = int(Tdec[g])
            h1 = (Kd + 1) // 2
            ends = ([p1 - 1] if Kd else []) + \
                   ([pos + h1 * 128 - 1] if 0 < h1 < Kd else [])
            for endslot in ends:
                if idx_s[core, endslot] >= 0 and idx_d[core, endslot] >= 0:
                    continue
                ok = np.nonzero((idx_s[core, pos:p1] >= 0) &
                                (idx_d[core, pos:p1] >= 0))[0] + pos
                ok = [j for j in ok if j not in ends]
                assert len(ok) > 0
                j = ok[0]
                for a2 in (idx_s, idx_d, slot2j):
                    a2[core, j], a2[core, endslot] = \
                        a2[core, endslot], a2[core, j]
            pos = p1
        assert pos == TOT_DEC

    xp = np.zeros((c.NP, c.D), dtype=np.float32)
    xp[:c.N] = np.asarray(x, dtype=np.float32)
    use_b1 = bool(np.any(np.asarray(b1)))
    use_b2 = bool(np.any(np.asarray(b2)))
    assert not (use_b1 or use_b2), "bias path not wired in v10"

    # xs = dinv * x, in band-major table order: layer-1 gathers read this
    # host-provided table directly -> no P0 / no table-1 collectives.
    xs = xp * dinv_f[:, None]
    perm = bmaj_of(np.arange(c.NP))        # node id -> table row
    xs_tab = np.zeros((c.NP, c.D), dtype=np.float32)
    xs_tab[perm] = xs
    xs_tab = np.ascontiguousarray(xs_tab).astype(BF)

    in_maps = []
    for core in range(c.NC):
        sl = slice(core * c.SHARD, (core + 1) * c.SHARD)
        dsh = dinv_f[sl]
        m = {
            "xs_tab": xs_tab,
            "xs_own": np.ascontiguousarray(
                xs[sl].reshape(c.NW, 128, c.D).transpose(1, 0, 2)
                .reshape(128, c.NW * c.D)).astype(BF),
            "W1": np.asarray(W1, dtype=np.float32).astype(BF),
            "W2": np.asarray(W2, dtype=np.float32).astype(BF),
            "dinv": np.ascontiguousarray(dsh.reshape(c.NW, 128).T),
            "gidx": _wrap_idxs(idx_arr[core]),
            "dstloc": np.ascontiguousarray(
                dloc_arr[core].reshape(TOT_TILES, 128).T).astype(BF),
            "didx_s": _wrap_idxs(idx_s[core]),
            "didx_d": _wrap_idxs(idx_d[core]),
        }
        in_maps.append(m)
    meta = dict(T=T, span_tiles=span_tiles, TOT=TOT, TOT_TILES=TOT_TILES,
                Tdec=Tdec, gorder=gorder, TOT_DEC=TOT_DEC, slot2j=slot2j,
                use_b1=use_b1, use_b2=use_b2)
    return in_maps, meta


def build_program(cfg, meta, num_cores=None):
    c = cfg
    NCores = num_cores or c.NC
    T, span_tiles = meta["T"], meta["span_tiles"]
    TOT, TOT_TILES = meta["TOT"], meta["TOT_TILES"]
    Tdec, gorder, TOT_DEC = meta["Tdec"], meta["gorder"], meta["TOT_DEC"]
    use_b = {1: meta["use_b1"], 2: meta["use_b2"]}
    D = c.D
    TBMAX = int(span_tiles.sum(axis=1).max())
    SPANMAX = int(span_tiles.max())
    DEC_SPANMAX = int(Tdec.max())
    GB_CHUNKS = c.GB_CHUNKS

    nc = bacc.Bacc("TRN2", target_bir_lowering=False, debug=False,
                   num_devices=NCores, num_swdge_queues=4)
    NQ = 4

    assert not (use_b[1] or use_b[2])
    xs_tab_in = nc.dram_tensor("xs_tab", [c.TB_STARTS[-1], D], BF16,
                               kind="ExternalInput")
    xs_own_in = nc.dram_tensor("xs_own", [128, c.SHARD], BF16,
                               kind="ExternalInput")
    W1_in = nc.dram_tensor("W1", [D, D], BF16, kind="ExternalInput")
    W2_in = nc.dram_tensor("W2", [D, D], BF16, kind="ExternalInput")
    dinv_in = nc.dram_tensor("dinv", [128, c.NW], F32, kind="ExternalInput")
    gidx_in = nc.dram_tensor("gidx", [128, TOT // 16], I16, kind="ExternalInput")
    dstloc_in = nc.dram_tensor("dstloc", [128, TOT_TILES], BF16, kind="ExternalInput")
    didx_s_in = nc.dram_tensor("didx_s", [128, TOT_DEC // 16], I16, kind="ExternalInput")
    didx_d_in = nc.dram_tensor("didx_d", [128, TOT_DEC // 16], I16, kind="ExternalInput")
    dots_out = nc.dram_tensor("dots", [128, TOT_DEC // 128], F32, kind="ExternalOutput")

    shard_b = {l: [nc.dram_tensor(f"shard{l}_{k}", [c.CH_SIZES[k], D], BF16)
                   for k in range(c.NB)] for l in (2, 3)}
    table = {l: nc.dram_tensor(f"table{l}", [c.TB_STARTS[-1], D], BF16,
                               addr_space="Shared") for l in (2, 3)}

    def tslice(l, g):
        """Gather source AP for band g: base at GBASE[g] (mid-band); signed
        int16 indices reach the whole band [BAND_LO, BAND_HI). Layer 1
        gathers the host-provided xs table (no collective needed)."""
        src = xs_tab_in if l == 1 else table[l]
        return src[c.GBASE[g]:c.BAND_HI[g], :]

    iota_dram = nc.inline_tensor(
        np.tile(np.arange(128, dtype=np.float32), (128, 1)).astype(BF), "iota_c")
    ident_dram = nc.inline_tensor(np.eye(128, dtype=np.float32).astype(BF), "ident_c")

    core_ids = list(range(NCores))
    gst = {"count": 0, "prev": None}
    ccst = {}                            # (l, K) -> collective instruction

    def emit_gather(out_ap, in_ap, idx_ap, n_idx, deps=()):
        q = gst["count"] % NQ
        inst = nc.gpsimd.dma_gather(out_ap, in_ap, idx_ap, n_idx, n_idx, D,
                                    queue_num=q, single_packet=False)
        for dcc in deps:
            add_dep_helper(inst.ins, dcc.ins, sync=True,
                           reason="gather after collective")
        gst["count"] += 1
        return inst

    def emit_collective(l, K):
        cc = nc.gpsimd.collective_compute(
            "AllGather", mybir.AluOpType.bypass,
            replica_groups=[core_ids],
            ins=[shard_b[l][K][:]],
            outs=[table[l][c.TB_STARTS[K]:c.TB_STARTS[K + 1], :]],
        )
        ccst[(l, K)] = cc
        return cc

    def write_window(l, w, src_ap, done_k):
        """DMA window w rows into its chunk shard; fire collectives when a
        chunk completes (chunks are window-aligned)."""
        lo = w * 128
        K = 0
        while c.CH_STARTS[K + 1] <= lo:
            K += 1
        off = lo - c.CH_STARTS[K]
        nc.sync.dma_start(shard_b[l][K][off:off + 128, :], src_ap)
        while len(done_k) < c.NB and \
                (w + 1) * 128 >= c.CH_STARTS[len(done_k) + 1]:
            emit_collective(l, len(done_k))
            done_k.append(len(done_k))

    with tile.TileContext(nc) as tc:
        with contextlib.ExitStack() as es:
            const = es.enter_context(tc.tile_pool(name="const", bufs=1))
            meta_p = es.enter_context(tc.tile_pool(name="meta", bufs=1))

            w1_sb = const.tile([D, D], BF16); nc.sync.dma_start(w1_sb[:], W1_in[:])
            w2_sb = const.tile([D, D], BF16); nc.sync.dma_start(w2_sb[:], W2_in[:])
            dinv_sb = const.tile([128, c.NW], F32)
            nc.sync.dma_start(dinv_sb[:], dinv_in[:])
            iota_sb = const.tile([128, 128], BF16)
            nc.sync.dma_start(iota_sb[:], iota_dram[:])
            ident_sb = const.tile([128, 128], BF16)
            nc.sync.dma_start(ident_sb[:], ident_dram[:])
            # gidx first: the first gathers need it; split the upload so the
            # first batches' indices land quickly
            gidx_sb = meta_p.tile([128, TOT // 16], I16)
            gsplit = min(TOT // 16, 1536)
            nc.scalar.dma_start(gidx_sb[:, :gsplit], gidx_in[:, :gsplit])
            nc.scalar.dma_start(gidx_sb[:, gsplit:], gidx_in[:, gsplit:])
            dstloc_sb = meta_p.tile([128, TOT_TILES], BF16)
            nc.scalar.dma_start(dstloc_sb[:], dstloc_in[:])
            xs_own_sb = meta_p.tile([128, c.SHARD], BF16)
            nc.sync.dma_start(xs_own_sb[:], xs_own_in[:])

            span_base = {}
            tcol0 = 0
            for b in range(c.NBATCH):
                m0 = 0
                for g in range(c.NGB):
                    span_base[(b, g)] = (tcol0, m0)
                    tcol0 += int(span_tiles[b, g])
                    m0 += int(span_tiles[b, g])

            pre = {}        # (lid, b) -> (dict g -> Mt, set of emitted gbands)
            cc_waited = {1: set(), 2: set(), 3: set()}
            SPANG = [int(span_tiles[:, g].max()) for g in range(c.NGB)]

            def batch_gathers(Mp, lid, b, only_avail=False):
                """Emit (remaining) gathers for batch b of layer lid. Per-band
                M pools: band A (early-available) pipelines deeply without
                waiting for band B's collectives."""
                Mts, done = pre.get((lid, b), ({}, set()))
                for g in range(c.NGB):
                    if g in done or int(span_tiles[b, g]) == 0:
                        continue
                    if lid > 1 and only_avail and any((lid, K) not in ccst
                                                      for K in GB_CHUNKS[g]):
                        continue
                    ntiles = int(span_tiles[b, g])
                    tb, _ = span_base[(b, g)]
                    deps = []
                    if lid > 1 and g not in cc_waited[lid]:
                        deps = [ccst[(lid, K)] for K in GB_CHUNKS[g]]
                        cc_waited[lid].add(g)
                    Mt = Mp.tile([128, SPANG[g], 128], BF16, tag=f"M{g}",
                                 bufs=(8 if g == 0 else 3))
                    # two half-gathers per span: spreads each batch across
                    # all 4 SWDGE queues (desc-gen parallelizes per queue)
                    h1 = (ntiles + 1) // 2
                    for lo, hi in ((0, h1), (h1, ntiles)):
                        if hi > lo:
                            emit_gather(
                                Mt[:, lo:hi, :], tslice(lid, g),
                                gidx_sb[:, (tb + lo) * 8:(tb + hi) * 8],
                                (hi - lo) * 128, deps=deps)
                    Mts[g] = Mt
                    done.add(g)
                pre[(lid, b)] = (Mts, done)
                return Mts

            def build_spans(Sp, b):
                """One wide DVE is_equal per (batch, band) span: all one-hot
                S tiles of the span in a single instruction."""
                Sw = {}
                for g in range(c.NGB):
                    K = int(span_tiles[b, g])
                    if K == 0:
                        continue
                    tb, mb = span_base[(b, g)]
                    St = Sp.tile([128, SPANG[g], 128], BF16, tag=f"S{g}",
                                 bufs=3)
                    nc.vector.tensor_tensor(
                        St[:, :K, :],
                        iota_sb[:].unsqueeze(1).broadcast_to([128, K, 128]),
                        dstloc_sb[:, tb:tb + K].unsqueeze(2)
                            .broadcast_to([128, K, 128]),
                        op=mybir.AluOpType.is_equal)
                    Sw[g] = (St, mb)
                return Sw

            def layer(Mp, lid, hall, hall_next, next_lid):
                """lid==1: aggregate raw xs, then apply W1 (transpose+matmul)
                and W2 per window to produce hs2. lid==2: aggregate hs2,
                relu+scale to produce z."""
                waited_done = []
                with tc.tile_pool(name=f"S{lid}", bufs=6) as Sp, \
                     tc.tile_pool(name=f"ag{lid}", bufs=4, space="PSUM") as agp, \
                     tc.tile_pool(name=f"tp{lid}", bufs=1, space="PSUM") as tpp, \
                     tc.tile_pool(name=f"ep{lid}", bufs=4) as epp:
                    for b in range(c.NBATCH):
                        wlo, whi = b * c.WB, min((b + 1) * c.WB, c.NW)
                        Mts = batch_gathers(Mp, lid, b)
                        Sw = build_spans(Sp, b)
                        for w in range(wlo, whi):
                            ps = agp.tile([128, D], F32, tag="agg")
                            nmm = int(T[w].sum())
                            hsl = hall[:, w * 128:w * 128 + D]
                            nc.tensor.matmul(ps[:], lhsT=ident_sb[:],
                                             rhs=hsl,
                                             start=True, stop=(nmm == 0))
                            mi = 0
                            for g in range(c.NGB):
                                if int(T[w, g]) == 0:
                                    continue
                                St, _ = Sw[g]
                                Mt = Mts[g]
                                off = int(T[wlo:w, g].sum())
                                for t in range(int(T[w, g])):
                                    k = off + t
                                    mi += 1
                                    nc.tensor.matmul(
                                        ps[:], lhsT=St[:, k, :],
                                        rhs=Mt[:, k, :],
                                        start=False,
                                        stop=(mi == nmm))
                            zo = None
                            if lid == 1:
                                # a1 = dinv*(agg+self)  [pre-W1 aggregate]
                                a1 = epp.tile([128, D], BF16, tag="a1")
                                nc.scalar.activation(
                                    a1[:], ps[:], ACTF.Copy,
                                    scale=dinv_sb[:, w:w + 1])
                                t1ps = tpp.tile([128, D], BF16, tag="t1")
                                nc.tensor.transpose(t1ps[:], a1[:], ident_sb[:])
                                a1T = epp.tile([128, D], BF16, tag="a1T")
                                nc.scalar.activation(a1T[:], t1ps[:], ACTF.Copy)
                                yps = tpp.tile([128, D], F32, tag="y")
                                nc.tensor.matmul(yps[:], lhsT=a1T[:],
                                                 rhs=w1_sb[:],
                                                 start=True, stop=True)
                                z = epp.tile([128, D], BF16, tag="z")
                                nc.scalar.activation(z[:], yps[:], ACTF.Relu)
                                t2ps = tpp.tile([128, D], BF16, tag="t2")
                                nc.tensor.transpose(t2ps[:], z[:], ident_sb[:])
                                zT = epp.tile([128, D], BF16, tag="zT")
                                nc.scalar.activation(zT[:], t2ps[:], ACTF.Copy)
                                h2ps = tpp.tile([128, D], F32, tag="h2")
                                nc.tensor.matmul(h2ps[:], lhsT=zT[:],
                                                 rhs=w2_sb[:],
                                                 start=True, stop=True)
                                nc.scalar.activation(
                                    hall_next[:, w * 128:w * 128 + D], h2ps[:],
                                    ACTF.Copy, scale=dinv_sb[:, w:w + 1])
                            else:
                                # layer-2 output only stages toward the shard
                                # DMA; use a small rolling buffer
                                zo = epp.tile([128, D], BF16, tag="zo")
                                nc.scalar.activation(
                                    zo[:], ps[:],
                                    ACTF.Relu, scale=dinv_sb[:, w:w + 1])
                            out_ap = (hall_next[:, w * 128:w * 128 + D]
                                      if lid == 1 else zo[:])
                            write_window(next_lid, w, out_ap, waited_done)
                        if b == c.NBATCH - 1 - c.PREFETCH and next_lid == 2:
                            for pb in range(min(c.PREFETCH + 1, c.NBATCH)):
                                batch_gathers(Mp, 2, pb, only_avail=True)

            with tc.tile_pool(name="hs", bufs=1) as hsp, \
                 tc.tile_pool(name="Mpool", bufs=1) as Mp:
                hs2_all = hsp.tile([128, c.SHARD], BF16, tag="hs2")
                layer(Mp, 1, xs_own_sb, hs2_all, 2)
                layer(Mp, 2, hs2_all, None, 3)

            # decode
            with tc.tile_pool(name="didx", bufs=1) as didxp, \
                 tc.tile_pool(name="dM", bufs=1) as dMp, \
                 tc.tile_pool(name="dw", bufs=6) as dwp, \
                 tc.tile_pool(name="dout", bufs=1) as doutp:
                ds_sb = didxp.tile([128, TOT_DEC // 16], I16)
                nc.scalar.dma_start(ds_sb[:], didx_s_in[:])
                dd_sb = didxp.tile([128, TOT_DEC // 16], I16)
                nc.scalar.dma_start(dd_sb[:], didx_d_in[:])
                Ms = dMp.tile([128, TOT_DEC // 128, D], BF16, tag="Ms")
                Md = dMp.tile([128, TOT_DEC // 128, D], BF16, tag="Md")
                res = doutp.tile([128, TOT_DEC // 128], F32)
                waited = set()
                coff = 0
                for g in gorder:
                    ks, kd = g // c.NGB, g % c.NGB
                    ncols = int(Tdec[g])
                    if ncols == 0:
                        continue
                    dep_s, dep_d = [], []
                    if ks not in waited:
                        dep_s = [ccst[(3, K)] for K in GB_CHUNKS[ks]]
                        waited.add(ks)
                    if kd not in waited:
                        dep_d = [ccst[(3, K)] for K in GB_CHUNKS[kd]]
                        waited.add(kd)
                    h1 = (ncols + 1) // 2
                    for lo, hi in ((0, h1), (h1, ncols)):
                        if hi <= lo:
                            continue
                        emit_gather(Ms[:, coff + lo:coff + hi, :],
                                    tslice(3, ks),
                                    ds_sb[:, (coff + lo) * 8:(coff + hi) * 8],
                                    (hi - lo) * 128, deps=dep_s)
                        emit_gather(Md[:, coff + lo:coff + hi, :],
                                    tslice(3, kd),
                                    dd_sb[:, (coff + lo) * 8:(coff + hi) * 8],
                                    (hi - lo) * 128, deps=dep_d)
                        dep_s, dep_d = [], []
                    # wide elementwise product + innermost-axis reduce: two
                    # DVE ops per group instead of per-tile mult+accum
                    mm = dwp.tile([128, DEC_SPANMAX, 128], F32, tag="mm",
                                  bufs=2)
                    nc.vector.tensor_tensor(
                        mm[:, :ncols, :], Ms[:, coff:coff + ncols, :],
                        Md[:, coff:coff + ncols, :],
                        op=mybir.AluOpType.mult)
                    nc.vector.tensor_reduce(
                        res[:, coff:coff + ncols], mm[:, :ncols, :],
                        axis=mybir.AxisListType.X, op=mybir.AluOpType.add)
                    coff += ncols
                nc.sync.dma_start(dots_out[:], res[:])

    nc.compile()
    return nc


def assemble_output(cfg, meta, results):
    c = cfg
    slot2j = meta["slot2j"]
    out = np.zeros(c.EL, dtype=np.float32)
    for core in range(len(results)):
        d = np.asarray(results[core]["dots"], dtype=np.float32)
        flat = d.T.reshape(-1)             # slot i -> d[i%128, i//128]
        s2j = slot2j[core]
        valid = s2j >= 0
        out[s2j[valid]] = flat[valid]
    return out


def run_pipeline(x, edge_index, edge_label_index, W1, b1, W2, b2,
                 cfg=None, trace=False, tmpdir=None):
    cfg = cfg or DEFAULT
    in_maps, meta = host_prep(cfg, x, edge_index, edge_label_index,
                              W1, b1, W2, b2)
    nc = build_program(cfg, meta)
    res = run_bass_kernel_spmd(nc, in_maps, list(range(cfg.NC)),
                               trace=trace, tmpdir=tmpdir)
    return assemble_output(cfg, meta, res.results), res


def kernel(x, edge_index, edge_label_index, W1, b1, W2, b2):
    out, _ = run_pipeline(x, edge_index, edge_label_index, W1, b1, W2, b2)
    return out
